# revision 28
# baseline (speedup 1.0000x reference)
"""Trainium2 Bass kernel for the gammatone-cochlea + LIF-SNN model.

Pipeline per core (32 of the 256 batch rows, pure data parallel):
  1. Gammatone conv [32ch, 64 taps] via tap-split Hankel matmuls (fp32 PE):
     4 batch rows per 128-partition group, block-diagonal lhsT, two
     accumulating matmuls per 512-sample block (taps 0-31 / 32-63, the
     second reading the same Hankel tile at free offset +32). One combined
     DMA per strip loads the 128-partition Hankel. The strip loop is
     TIME-MAJOR (strip index outer, group inner) so that after strip si
     every batch row's envelope is known for t < 16*si + 15.
  2. ReLU on ScalarE (PSUM -> SBUF copy), then DVE strided block-sums
     (128-sample blocks) into S_all. The /256 framing normalization is
     folded into the AN scales.
  3. Per strip: one strided DVE add forms the two-block sums for the new
     t-window across all 8 groups, selector matmuls replicate channels
     4x across partitions (u,c), and the AN stage (fused tensor_scalar
     mult+is_gt, 3 accumulating matmuls) produces bushy currents which
     land in Q[0:50] at columns b*136 + 1 + t.
  4. SNN: one 32-row packed wavefront. All three LIF layers (bushy 50,
     IC at partitions 64-113, AC at 114-123) update in [124,32] DVE ops:
     scalar_tensor_tensor (beta*mem + cur), is_gt spike, subtract reset.
     IC lags bushy by DELTA steps and AC by 2*DELTA; the per-step PE
     matmul blockdiag(WicT,WacT) @ [spk_b; spk_ic] and its ScalarE
     PSUM->SBUF copy have DELTA steps of slack. Wavefront steps are
     paced through the conv loop, so only ~23 steps remain as tail.
Outputs: spikes/membranes live in partitions 114-123 of the state tiles,
columns b*136 + t + 1 + 2*DELTA; host slices + transposes.
"""
import numpy as np
import concourse.bass as bass
import concourse.bacc as bacc
import concourse.mybir as mybir
import concourse.tile as tile
from concourse.bass_utils import run_bass_kernel_spmd

dt = mybir.dt
AF = mybir.ActivationFunctionType
OP = mybir.AluOpType

NCORES = 8
B, N, C, K = 256, 16000, 32, 64
BLOC = B // NCORES            # 32 batch rows per core
WINDOW, STRIDE, T = 256, 128, 124
ANS, HID, OUT = 10, 50, 10
BETA, THR, AN_THR = 0.95, 1.0, 0.5
PAD_L, PAD_R = 31, 33         # SAME padding for K=64: 31 left, 32 right (+1 slack)
NPAD = PAD_L + N + PAD_R      # 16064
NGRP = BLOC // 4              # 8 groups of 4 rows
NSTRIP = 8
STRIPS = [2048] * 7 + [1664]  # 4-block strips per group

# packed-wavefront SNN layout (partition-sliced ops must start at a
# quadrant boundary, so IC sits at partitions 64-113, AC at 114-123, and
# partitions 50-63 stay zero)
DELTA = 2                     # IC lags bushy by DELTA steps, AC by 2*DELTA
PIC, PTOT = 64, 124           # IC partition base; total SNN partitions
BST = 132                     # per-batch-row column stride (1 + 124 + 2*DELTA <= BST)
FREE = BLOC * BST             # 4352
NSTEP = T + 1 + 2 * DELTA     # wavefront steps tau = 1..NSTEP (130)
OFF0 = 1 + 2 * DELTA          # AC output for t sits at column b*BST + OFF0 + t

# jnp.linspace(0.5, 1.5, 10, dtype=f32), bitexact
_SCALES = np.array([0x3F000000, 0x3F1C71C7, 0x3F38E38E, 0x3F555555, 0x3F71C71D,
                    0x3F871C72, 0x3F955556, 0x3FA38E39, 0x3FB1C71D, 0x3FC00000],
                   dtype=np.uint32).view(np.float32)

_NC_CACHE = None


def _build_nc():
    nc = bacc.Bacc("TRN2", target_bir_lowering=False, debug=False,
                   num_devices=NCORES)

    apadh = nc.dram_tensor("apadh", [BLOC, NPAD], dt.float16,
                           kind="ExternalInput")
    apadl = nc.dram_tensor("apadl", [BLOC, NPAD], dt.float16,
                           kind="ExternalInput")
    lw = nc.dram_tensor("lw", [6, 128, 128], dt.float16, kind="ExternalInput")
    wb = nc.dram_tensor("wb", [6, 128, HID], dt.float16, kind="ExternalInput")
    wca = nc.dram_tensor("wca", [PIC + HID, HID + OUT], dt.float32,
                         kind="ExternalInput")
    sv = nc.dram_tensor("sv", [128, 3], dt.float32, kind="ExternalInput")
    selr = nc.dram_tensor("selr", [4, 128, 128], dt.float32, kind="ExternalInput")
    ospk = nc.dram_tensor("ospk", [OUT, FREE], dt.float32, kind="ExternalOutput")
    omem = nc.dram_tensor("omem", [OUT, FREE], dt.float32, kind="ExternalOutput")

    with tile.TileContext(nc) as tc:
        with tc.tile_pool(name="cpool", bufs=1) as cp:
            # stationary conv operands (fp16): per tap-half {2^11*kh,
            # 2^11*kl, kh}; paired with moving {ah, ah, 2^11*al}
            lwt = [cp.tile([128, 128], dt.float16, name=f"lw{i}")
                   for i in range(6)]
            for i in range(6):
                nc.gpsimd.dma_start(out=lwt[i][:, :], in_=lw[i, :, :])
            svt = cp.tile([128, 3], dt.float32)
            wbt = [cp.tile([128, HID], dt.float16, name=f"wbt{i}") for i in range(6)]
            wcat = cp.tile([PIC + HID, HID + OUT], dt.float32)
            selt = [cp.tile([128, 128], dt.float32, name=f"selt{r}")
                    for r in range(4)]

            def load_aux_weights():
                # deferred until after si=0's Hankel DMAs: none of these are
                # consumed before the si=1 an_window / wavefront stages
                nc.gpsimd.dma_start(out=svt[:, :], in_=sv[:, :])
                for i in range(6):
                    nc.gpsimd.dma_start(out=wbt[i][:, :], in_=wb[i, :, :])
                nc.gpsimd.dma_start(out=wcat[:, :], in_=wca[:, :])
                for r in range(4):
                    nc.gpsimd.dma_start(out=selt[r][:, :], in_=selr[r, :, :])

            S_all = cp.tile([128, NGRP * 126], dt.float32)

            # packed SNN state (32 rows wide)
            Mt = cp.tile([PTOT, FREE], dt.float32, name="Mt")
            St = cp.tile([PTOT, FREE], dt.float32, name="St")
            Qb = cp.tile([HID, FREE], dt.float32, name="Qb")
            nc.gpsimd.memset(Mt[:, :], 0.0)
            nc.gpsimd.memset(St[:, :], 0.0)
            nc.gpsimd.memset(Qb[:, :], 0.0)

            hkp = tc.alloc_tile_pool(name="hkp", bufs=8)
            ybp = tc.alloc_tile_pool(name="ybp", bufs=8)
            anp = tc.alloc_tile_pool(name="anp", bufs=2)
            sep = tc.alloc_tile_pool(name="sep", bufs=2)
            e4p = tc.alloc_tile_pool(name="e4p", bufs=2)
            qip = tc.alloc_tile_pool(name="qip", bufs=DELTA + 3)
            psp = tc.alloc_tile_pool(name="psp", bufs=1, space="PSUM")
            pss = tc.alloc_tile_pool(name="pss", bufs=1, space="PSUM")
            psn = tc.alloc_tile_pool(name="psn", bufs=1, space="PSUM")

            # PE clock warm-up: one throwaway matmul as soon as the first
            # stationary tile lands (~0.4us) starts the HAM p-state ramp,
            # so conv matmuls reach full clock ~2us earlier
            wps = pss.tile([128, 128], dt.float32, tag="win", bufs=1,
                           name="warm")
            nc.tensor.matmul(wps[:, :], lwt[0][:, :], lwt[0][:, :],
                             start=True, stop=True)

            strip_ctr = [0]

            def conv_strip(g, si, piece=None):
                """Conv + framing for rows 4g..4g+4, samples 2048si..+sw."""
                sc = strip_ctr[0]
                strip_ctr[0] += 1
                if piece is None:
                    sw = STRIPS[si]
                    s0 = 2048 * si
                else:
                    s0, sw = piece
                hkh = hkp.tile([128, 2112], dt.float16, tag="hkh", name="hkh")
                hkl = hkp.tile([128, 2112], dt.float16, tag="hkl", name="hkl")
                # Hankel: hk[32*r+k, j] = apad[4g+r, s0 + j + k], one DMA each
                srch = bass.AP(apadh, (4 * g) * NPAD + s0,
                               [[NPAD, 4], [1, 32], [1, sw + 32]])
                nc.sync.dma_start(out=hkh[:, 0:sw + 32], in_=srch)
                srcl = bass.AP(apadl, (4 * g) * NPAD + s0,
                               [[NPAD, 4], [1, 32], [1, sw + 32]])
                nc.sync.dma_start(out=hkl[:, 0:sw + 32], in_=srcl)
                nb4 = (sw + 511) // 512
                accs = []
                # fp16 hi/lo split, 2^11-scaled PSUM: for each tap half,
                # accumulate ah*(2^11 kh) + ah*(2^11 kl) + (2^11 al)*kh;
                # the al*kl term is below fp32 rounding. The 2^-11 is
                # folded into the AN scales (relu commutes with it).
                passes = [(0, hkh, 0), (1, hkh, 0), (2, hkl, 0),
                          (3, hkh, 32), (4, hkh, 32), (5, hkl, 32)]
                for b4 in range(nb4):
                    w = min(512, sw - 512 * b4)
                    acc = psp.tile([128, 512], dt.float32,
                                   tag=f"acc{(4 * sc + b4) % 5}", name="acc")
                    accs.append((acc, w))
                for pi, (li, hkt, off) in enumerate(passes):
                    for b4 in range(nb4):
                        acc, w = accs[b4]
                        nc.tensor.matmul(
                            acc[:, 0:w], lwt[li][:, :],
                            hkt[:, 512 * b4 + off:512 * b4 + off + w],
                            start=(pi == 0), stop=(pi == 5))
                for b4 in range(nb4):
                    acc, w = accs[b4]
                    yb = ybp.tile([128, 512], dt.float32, tag="yb", name="yb")
                    nc.scalar.activation(yb[:, 0:w], acc[:, 0:w], AF.Relu)
                    nblk = w // 128
                    i = s0 // 512 + b4
                    view = bass.AP(yb.tensor, yb.offset,
                                   [list(yb.ap[0]), [128, nblk], [1, 128]])
                    nc.vector.tensor_reduce(
                        S_all[:, g * 126 + 4 * i: g * 126 + 4 * i + nblk],
                        view, axis=mybir.AxisListType.X, op=OP.add)

            def _win(si):
                t0 = max(0, 16 * si - 1)
                t1 = min(T - 1, 16 * si + 14)
                return t0, t1 - t0 + 1

            anw_state = {}

            def anw_se(si, tw=None):
                """Two-block sums for the t-window unlocked by strip si:
                t in [max(0,16si-1), min(123,16si+14)], cols (g, t)."""
                t0, W = tw if tw is not None else _win(si)
                se = sep.tile([128, NGRP * W], dt.float32, tag="senv",
                              name="senv")
                sa = S_all[:, :]
                in0 = bass.AP(sa.tensor, sa.offset + t0,
                              [list(sa.ap[0]), [126, NGRP], [1, W]])
                in1 = bass.AP(sa.tensor, sa.offset + t0 + 1,
                              [list(sa.ap[0]), [126, NGRP], [1, W]])
                nc.vector.tensor_tensor(se[:, :], in0, in1, OP.add)
                anw_state[si] = se

            def anw_shf(si, tw=None):
                """Channel 4x replication via selector matmuls: psum cols
                (r, g, t); Act copy reorders to (g, r, t) = (b, t)."""
                t0, W = tw if tw is not None else _win(si)
                se = anw_state.pop(si)
                shf = pss.tile([128, 4 * NGRP * W], dt.float32, tag="win",
                               bufs=1, name="shf")
                for r in range(4):
                    nc.tensor.matmul(shf[:, r * NGRP * W:(r + 1) * NGRP * W],
                                     selt[r][:, :], se[:, :],
                                     start=True, stop=True)
                e4 = e4p.tile([128, 4 * NGRP * W], dt.float32, tag="e4",
                              name="e4")
                sh = shf[:, :]
                dst = bass.AP(e4.tensor, e4.offset,
                              [list(e4.ap[0]), [W, 4], [4 * W, NGRP], [1, W]])
                nc.scalar.activation(dst, sh, AF.Copy)
                anw_state[si] = e4

            def anw_an(si, tw=None):
                """AN spikes + bushy currents, cols (b, t) -> Qb columns."""
                t0, W = tw if tw is not None else _win(si)
                e4 = anw_state.pop(si)
                # fp16 hi/lo split of W_bushy: spikes (0/1) and 2^-11 are
                # fp16-exact, so cur_b = Wbh@spk + Wbl_s@(2^-11 spk) matches
                # fp32 up to a 2^-22 residual (verified: 0 spike flips)
                ps_cb = pss.tile([HID, 4 * NGRP * W], dt.float32, tag="win",
                                 bufs=1, name="ps_cb")
                for ch in range(3):
                    an = anp.tile([128, 4 * NGRP * W], dt.float16, tag="an",
                                  name="an")
                    nc.vector.tensor_scalar(an[:, :], e4[:, :],
                                            svt[:, ch:ch + 1], AN_THR,
                                            OP.mult, OP.is_gt)
                    an2 = anp.tile([128, 4 * NGRP * W], dt.float16, tag="an2",
                                   name="an2")
                    nc.vector.tensor_scalar(an2[:, :], an[:, :],
                                            1.0 / 2048.0, None, OP.mult)
                    nc.tensor.matmul(ps_cb[:, :], wbt[2 * ch][:, :], an[:, :],
                                     start=(ch == 0), stop=False)
                    nc.tensor.matmul(ps_cb[:, :], wbt[2 * ch + 1][:, :],
                                     an2[:, :],
                                     start=False, stop=(ch == 2))
                qap = Qb[:, :]
                dest = bass.AP(qap.tensor, qap.offset + t0 + 1,
                               [list(qap.ap[0]), [BST, BLOC], [1, W]])
                nc.scalar.activation(dest, ps_cb[:, :], AF.Copy)

            def tsl(ap2d, tau):
                # strided time-slice: columns b*BST + tau for b in 0..BLOC
                return bass.AP(ap2d.tensor, ap2d.offset + tau,
                               [list(ap2d.ap[0]), [BST, BLOC]])

            def wavefront():
                """Generator: packed LIF wavefront (32 rows), yields per step."""
                m_all, s_all = Mt[:, :], St[:, :]
                m_lo, m_hi = Mt[0:HID, :], Mt[PIC:PTOT, :]
                s_mm = St[0:PIC + HID, :]
                q_all = Qb[:, :]
                qia = {}
                for tau in range(1, NSTEP + 1):
                    nc.vector.scalar_tensor_tensor(
                        tsl(m_lo, tau), tsl(m_lo, tau - 1), BETA,
                        tsl(q_all, tau), OP.mult, OP.add)
                    if tau <= DELTA:
                        nc.vector.tensor_scalar(tsl(m_hi, tau),
                                                tsl(m_hi, tau - 1), BETA,
                                                None, OP.mult)
                    else:
                        qt = qia.pop(tau)
                        nc.vector.scalar_tensor_tensor(
                            tsl(m_hi, tau), tsl(m_hi, tau - 1), BETA,
                            qt[PIC:PTOT, :], OP.mult, OP.add)
                    nc.vector.tensor_scalar(tsl(s_all, tau), tsl(m_all, tau),
                                            THR, None, OP.is_gt)
                    if tau + DELTA <= NSTEP:
                        ps = psn.tile([PTOT, BLOC], dt.float32, tag="snn",
                                      bufs=2, name="ps_snn")
                        nc.tensor.matmul(ps[PIC:PTOT, :], wcat[:, :],
                                         tsl(s_mm, tau),
                                         start=True, stop=True)
                        qt = qip.tile([PTOT, BLOC], dt.float32, tag="qia",
                                      name="qia")
                        qia[tau + DELTA] = qt
                        nc.scalar.activation(qt[PIC:PTOT, :], ps[PIC:PTOT, :],
                                             AF.Copy)
                    nc.vector.tensor_tensor(tsl(m_all, tau), tsl(m_all, tau),
                                            tsl(s_all, tau), OP.subtract)
                    yield

            wf = wavefront()
            emitted = [0]

            def pump(upto):
                upto = min(upto, NSTEP)
                while emitted[0] < upto:
                    next(wf)
                    emitted[0] += 1

            # an_window for strip si-1 is pipelined across the strips of
            # si (se add at g==1, selector matmuls at g==2, AN at g==3) so
            # the PE never waits on the cross-engine envelope chain.
            for si in range(NSTRIP):
                for g in range(NGRP):
                    if si == 0:
                        # narrow pieces: the PE can start after ~0.8us of
                        # DMA instead of 3us, shortening the p-state ramp
                        if g == 0:
                            for p4 in range(4):
                                conv_strip(g, 0, piece=(512 * p4, 512))
                            load_aux_weights()
                        else:
                            conv_strip(g, 0, piece=(0, 1024))
                            conv_strip(g, 0, piece=(1024, 1024))
                        continue
                    conv_strip(g, si)
                    if si >= 1:
                        if g == 1:
                            anw_se(si - 1)
                        elif g == 2:
                            anw_shf(si - 1)
                        elif g == 3:
                            anw_an(si - 1)
                    # pace wavefront steps unlocked by window si-2 early in
                    # this strip row, window si-1 once anw_an(si-1) ran
                    if g < 4:
                        pump(16 * si - 33 + 4 * (g + 1))
                    else:
                        pump(16 * si - 17 + 4 * (g - 3))
            # final window in two t-chunks so the tail wavefront overlaps
            # the second chunk's envelope/AN chain
            for ck, tw in enumerate(((111, 5), (116, 5), (121, 3))):
                key = (7, ck)
                anw_se(key, tw)
                anw_shf(key, tw)
                anw_an(key, tw)
                pump(tw[0] + tw[1])
            pump(NSTEP)

            nc.sync.dma_start(out=ospk[:, :], in_=St[PIC + HID:PTOT, :])
            nc.sync.dma_start(out=omem[:, :], in_=Mt[PIC + HID:PTOT, :])

            psn.release()
            pss.release()
            psp.release()
            qip.release()
            e4p.release()
            sep.release()
            anp.release()
            ybp.release()
            hkp.release()

    nc.finalize()
    return nc


def _prep_inputs(audio, gt_kernels, W_bushy, W_ic, W_ac):
    audio = np.ascontiguousarray(audio, dtype=np.float32)
    gt = np.ascontiguousarray(gt_kernels, dtype=np.float32)
    Wb = np.ascontiguousarray(W_bushy, dtype=np.float32)

    gth = gt.astype(np.float16)
    gtl = ((gt - gth.astype(np.float32)) * 2048.0).astype(np.float16)
    lw = np.zeros((6, 128, 128), np.float16)
    for r in range(4):
        sl = slice(r * 32, r * 32 + 32)
        # lhsT[r*32+k, r*32+c] = gt[c, k]; order: {2^11 kh, 2^11 kl, kh}
        # per tap half
        lw[0, sl, sl] = (gth[:, 0:32].astype(np.float32).T * 2048.0
                         ).astype(np.float16)
        lw[1, sl, sl] = gtl[:, 0:32].T
        lw[2, sl, sl] = gth[:, 0:32].T
        lw[3, sl, sl] = (gth[:, 32:64].astype(np.float32).T * 2048.0
                         ).astype(np.float16)
        lw[4, sl, sl] = gtl[:, 32:64].T
        lw[5, sl, sl] = gth[:, 32:64].T

    wb32 = np.zeros((3, 128, HID), np.float32)
    sv = np.zeros((128, 3), np.float32)
    for ch in range(3):
        for u in range(4):
            a = ch * 4 + u
            if a >= ANS:
                continue
            # wb32[ch, u*32+c, h] = W_bushy[h, c*10 + a]
            wb32[ch, u * 32:u * 32 + 32, :] = Wb[:, a::ANS].T
            # AN consumes raw 2^11-scaled two-block sums: fold the /256
            # and the conv 2^-11 into the scale (exact powers of two)
            sv[u * 32:u * 32 + 32, ch] = _SCALES[a] / (256.0 * 2048.0)
    wb = np.zeros((6, 128, HID), np.float16)
    for ch in range(3):
        wbh = wb32[ch].astype(np.float16)
        wb[2 * ch] = wbh
        wb[2 * ch + 1] = ((wb32[ch] - wbh.astype(np.float32)) * 2048.0
                          ).astype(np.float16)
    selr = np.zeros((4, 128, 128), np.float32)
    for r in range(4):
        for u in range(4):
            for c in range(32):
                selr[r, r * 32 + c, u * 32 + c] = 1.0
    # combined IC/AC lhsT: out partitions 0-49 = IC currents from spk_b
    # (spikes at partitions 0-49), 50-59 = AC currents from spk_ic (spikes
    # at partitions 64-113); contraction rows 50-63 are zero
    wca = np.zeros((PIC + HID, HID + OUT), np.float32)
    wca[0:HID, 0:HID] = np.ascontiguousarray(W_ic.T, dtype=np.float32)
    wca[PIC:PIC + HID, HID:HID + OUT] = np.ascontiguousarray(W_ac.T,
                                                             dtype=np.float32)

    apad = np.zeros((B, NPAD), np.float32)
    apad[:, PAD_L:PAD_L + N] = audio
    apadh = apad.astype(np.float16)
    apadl = ((apad - apadh.astype(np.float32)) * 2048.0).astype(np.float16)

    in_maps = []
    for c in range(NCORES):
        rows = slice(c * BLOC, (c + 1) * BLOC)
        in_maps.append({"apadh": apadh[rows], "apadl": apadl[rows],
                        "lw": lw, "wb": wb,
                        "wca": wca, "sv": sv, "selr": selr})
    return in_maps


def kernel(audio, gt_kernels, W_bushy, W_ic, W_ac, _trace=False):
    global _NC_CACHE
    if _NC_CACHE is None:
        _NC_CACHE = _build_nc()
    nc = _NC_CACHE
    in_maps = _prep_inputs(audio, gt_kernels, W_bushy, W_ic, W_ac)
    res = run_bass_kernel_spmd(nc, in_maps, core_ids=list(range(NCORES)),
                               trace=_trace)
    spk = np.empty((B, T, OUT), np.float32)
    mem = np.empty((B, T, OUT), np.float32)
    for c in range(NCORES):
        rs = res.results[c]["ospk"].reshape(OUT, BLOC, BST)
        rm = res.results[c]["omem"].reshape(OUT, BLOC, BST)
        rows = slice(c * BLOC, (c + 1) * BLOC)
        # [o, b, tau] -> [b, t, o] with t = tau - OFF0
        spk[rows] = rs[:, :, OFF0:OFF0 + T].transpose(1, 2, 0)
        mem[rows] = rm[:, :, OFF0:OFF0 + T].transpose(1, 2, 0)
    kernel._last_results = res
    return spk, mem


# revision 31
# speedup vs baseline: 1.0134x; 1.0134x over previous
"""Trainium2 Bass kernel for the gammatone-cochlea + LIF-SNN model.

Pipeline per core (32 of the 256 batch rows, pure data parallel):
  1. Gammatone conv [32ch, 64 taps] via tap-split Hankel matmuls (fp32 PE):
     4 batch rows per 128-partition group, block-diagonal lhsT, two
     accumulating matmuls per 512-sample block (taps 0-31 / 32-63, the
     second reading the same Hankel tile at free offset +32). One combined
     DMA per strip loads the 128-partition Hankel. The strip loop is
     TIME-MAJOR (strip index outer, group inner) so that after strip si
     every batch row's envelope is known for t < 16*si + 15.
  2. ReLU on ScalarE (PSUM -> SBUF copy), then DVE strided block-sums
     (128-sample blocks) into S_all. The /256 framing normalization is
     folded into the AN scales.
  3. Per strip: one strided DVE add forms the two-block sums for the new
     t-window across all 8 groups, selector matmuls replicate channels
     4x across partitions (u,c), and the AN stage (fused tensor_scalar
     mult+is_gt, 3 accumulating matmuls) produces bushy currents which
     land in Q[0:50] at columns b*136 + 1 + t.
  4. SNN: one 32-row packed wavefront. All three LIF layers (bushy 50,
     IC at partitions 64-113, AC at 114-123) update in [124,32] DVE ops:
     scalar_tensor_tensor (beta*mem + cur), is_gt spike, subtract reset.
     IC lags bushy by DELTA steps and AC by 2*DELTA; the per-step PE
     matmul blockdiag(WicT,WacT) @ [spk_b; spk_ic] and its ScalarE
     PSUM->SBUF copy have DELTA steps of slack. Wavefront steps are
     paced through the conv loop, so only ~23 steps remain as tail.
Outputs: spikes/membranes live in partitions 114-123 of the state tiles,
columns b*136 + t + 1 + 2*DELTA; host slices + transposes.
"""
import numpy as np
import concourse.bass as bass
import concourse.bacc as bacc
import concourse.mybir as mybir
import concourse.tile as tile
from concourse.bass_utils import run_bass_kernel_spmd

dt = mybir.dt
AF = mybir.ActivationFunctionType
OP = mybir.AluOpType

NCORES = 8
B, N, C, K = 256, 16000, 32, 64
BLOC = B // NCORES            # 32 batch rows per core
WINDOW, STRIDE, T = 256, 128, 124
ANS, HID, OUT = 10, 50, 10
BETA, THR, AN_THR = 0.95, 1.0, 0.5
PAD_L, PAD_R = 31, 33         # SAME padding for K=64: 31 left, 32 right (+1 slack)
NPAD = PAD_L + N + PAD_R      # 16064
NGRP = BLOC // 4              # 8 groups of 4 rows
NSTRIP = 8
STRIPS = [2048] * 7 + [1664]  # 4-block strips per group

# packed-wavefront SNN layout (partition-sliced ops must start at a
# quadrant boundary, so IC sits at partitions 64-113, AC at 114-123, and
# partitions 50-63 stay zero)
DELTA = 2                     # IC lags bushy by DELTA steps, AC by 2*DELTA
PIC, PTOT = 64, 124           # IC partition base; total SNN partitions
BST = 132                     # per-batch-row column stride (1 + 124 + 2*DELTA <= BST)
FREE = BLOC * BST             # 4352
NSTEP = T + 1 + 2 * DELTA     # wavefront steps tau = 1..NSTEP (130)
OFF0 = 1 + 2 * DELTA          # AC output for t sits at column b*BST + OFF0 + t

# jnp.linspace(0.5, 1.5, 10, dtype=f32), bitexact
_SCALES = np.array([0x3F000000, 0x3F1C71C7, 0x3F38E38E, 0x3F555555, 0x3F71C71D,
                    0x3F871C72, 0x3F955556, 0x3FA38E39, 0x3FB1C71D, 0x3FC00000],
                   dtype=np.uint32).view(np.float32)

_NC_CACHE = None


def _build_nc():
    nc = bacc.Bacc("TRN2", target_bir_lowering=False, debug=False,
                   num_devices=NCORES)

    apadh = nc.dram_tensor("apadh", [BLOC, NPAD], dt.float16,
                           kind="ExternalInput")
    apadl = nc.dram_tensor("apadl", [BLOC, NPAD], dt.float16,
                           kind="ExternalInput")
    lw = nc.dram_tensor("lw", [6, 128, 128], dt.float16, kind="ExternalInput")
    wb = nc.dram_tensor("wb", [6, 128, HID], dt.float16, kind="ExternalInput")
    wca = nc.dram_tensor("wca", [PIC + HID, HID + OUT], dt.float32,
                         kind="ExternalInput")
    sv = nc.dram_tensor("sv", [128, 3], dt.float32, kind="ExternalInput")
    selr = nc.dram_tensor("selr", [4, 128, 128], dt.float32, kind="ExternalInput")
    ospk = nc.dram_tensor("ospk", [OUT, FREE], dt.float32, kind="ExternalOutput")
    omem = nc.dram_tensor("omem", [OUT, FREE], dt.float32, kind="ExternalOutput")

    with tile.TileContext(nc) as tc:
        with tc.tile_pool(name="cpool", bufs=1) as cp:
            # stationary conv operands (fp16): per tap-half {2^11*kh,
            # 2^11*kl, kh}; paired with moving {ah, ah, 2^11*al}
            lwt = [cp.tile([128, 128], dt.float16, name=f"lw{i}")
                   for i in range(6)]
            for i in range(6):
                nc.gpsimd.dma_start(out=lwt[i][:, :], in_=lw[i, :, :])
            svt = cp.tile([128, 3], dt.float32)
            wbt = [cp.tile([128, HID], dt.float16, name=f"wbt{i}") for i in range(6)]
            wcat = cp.tile([PIC + HID, HID + OUT], dt.float32)
            selt = [cp.tile([128, 128], dt.float32, name=f"selt{r}")
                    for r in range(4)]

            def load_aux_weights():
                # deferred until after si=0's Hankel DMAs: none of these are
                # consumed before the si=1 an_window / wavefront stages
                nc.gpsimd.dma_start(out=svt[:, :], in_=sv[:, :])
                for i in range(6):
                    nc.gpsimd.dma_start(out=wbt[i][:, :], in_=wb[i, :, :])
                nc.gpsimd.dma_start(out=wcat[:, :], in_=wca[:, :])
                for r in range(4):
                    nc.gpsimd.dma_start(out=selt[r][:, :], in_=selr[r, :, :])

            S_all = cp.tile([128, NGRP * 126], dt.float32)

            # packed SNN state (32 rows wide)
            Mt = cp.tile([PTOT, FREE], dt.float32, name="Mt")
            St = cp.tile([PTOT, FREE], dt.float32, name="St")
            Qb = cp.tile([HID, FREE], dt.float32, name="Qb")
            nc.gpsimd.memset(Mt[:, :], 0.0)
            nc.gpsimd.memset(St[:, :], 0.0)
            nc.gpsimd.memset(Qb[:, :], 0.0)

            hkp = tc.alloc_tile_pool(name="hkp", bufs=8)
            ybp = tc.alloc_tile_pool(name="ybp", bufs=8)
            anp = tc.alloc_tile_pool(name="anp", bufs=2)
            sep = tc.alloc_tile_pool(name="sep", bufs=2)
            e4p = tc.alloc_tile_pool(name="e4p", bufs=2)
            qip = tc.alloc_tile_pool(name="qip", bufs=DELTA + 3)
            psp = tc.alloc_tile_pool(name="psp", bufs=1, space="PSUM")
            pss = tc.alloc_tile_pool(name="pss", bufs=1, space="PSUM")
            psn = tc.alloc_tile_pool(name="psn", bufs=1, space="PSUM")

            # PE clock warm-up: one throwaway matmul as soon as the first
            # stationary tile lands (~0.4us) starts the HAM p-state ramp,
            # so conv matmuls reach full clock ~2us earlier
            wps = pss.tile([128, 128], dt.float32, tag="win", bufs=1,
                           name="warm")
            nc.tensor.matmul(wps[:, :], lwt[0][:, :], lwt[0][:, :],
                             start=True, stop=True)

            strip_ctr = [0]

            def conv_strip(g, si, piece=None):
                """Conv + framing for rows 4g..4g+4, samples 2048si..+sw."""
                sc = strip_ctr[0]
                strip_ctr[0] += 1
                if piece is None:
                    sw = STRIPS[si]
                    s0 = 2048 * si
                else:
                    s0, sw = piece
                hkh = hkp.tile([128, 2112], dt.float16, tag="hkh", name="hkh")
                hkl = hkp.tile([128, 2112], dt.float16, tag="hkl", name="hkl")
                # Hankel: hk[32*r+k, j] = apad[4g+r, s0 + j + k], one DMA each
                srch = bass.AP(apadh, (4 * g) * NPAD + s0,
                               [[NPAD, 4], [1, 32], [1, sw + 32]])
                nc.sync.dma_start(out=hkh[:, 0:sw + 32], in_=srch)
                srcl = bass.AP(apadl, (4 * g) * NPAD + s0,
                               [[NPAD, 4], [1, 32], [1, sw + 32]])
                nc.sync.dma_start(out=hkl[:, 0:sw + 32], in_=srcl)
                nb4 = (sw + 511) // 512
                accs = []
                # fp16 hi/lo split, 2^11-scaled PSUM: for each tap half,
                # accumulate ah*(2^11 kh) + ah*(2^11 kl) + (2^11 al)*kh;
                # the al*kl term is below fp32 rounding. The 2^-11 is
                # folded into the AN scales (relu commutes with it).
                passes = [(0, hkh, 0), (1, hkh, 0), (2, hkl, 0),
                          (3, hkh, 32), (4, hkh, 32), (5, hkl, 32)]
                for b4 in range(nb4):
                    w = min(512, sw - 512 * b4)
                    acc = psp.tile([128, 512], dt.float32,
                                   tag=f"acc{(4 * sc + b4) % 5}", name="acc")
                    accs.append((acc, w))
                for pi, (li, hkt, off) in enumerate(passes):
                    for b4 in range(nb4):
                        acc, w = accs[b4]
                        nc.tensor.matmul(
                            acc[:, 0:w], lwt[li][:, :],
                            hkt[:, 512 * b4 + off:512 * b4 + off + w],
                            start=(pi == 0), stop=(pi == 5))
                for b4 in range(nb4):
                    acc, w = accs[b4]
                    yb = ybp.tile([128, 512], dt.float32, tag="yb", name="yb")
                    nc.scalar.activation(yb[:, 0:w], acc[:, 0:w], AF.Relu)
                    nblk = w // 128
                    i = s0 // 512 + b4
                    view = bass.AP(yb.tensor, yb.offset,
                                   [list(yb.ap[0]), [128, nblk], [1, 128]])
                    nc.vector.tensor_reduce(
                        S_all[:, g * 126 + 4 * i: g * 126 + 4 * i + nblk],
                        view, axis=mybir.AxisListType.X, op=OP.add)

            def _win(si):
                t0 = max(0, 16 * si - 1)
                t1 = min(T - 1, 16 * si + 14)
                return t0, t1 - t0 + 1

            anw_state = {}

            def anw_se(si, tw=None):
                """Two-block sums for the t-window unlocked by strip si:
                t in [max(0,16si-1), min(123,16si+14)], cols (g, t)."""
                t0, W = tw if tw is not None else _win(si)
                se = sep.tile([128, NGRP * W], dt.float32, tag="senv",
                              name="senv")
                sa = S_all[:, :]
                in0 = bass.AP(sa.tensor, sa.offset + t0,
                              [list(sa.ap[0]), [126, NGRP], [1, W]])
                in1 = bass.AP(sa.tensor, sa.offset + t0 + 1,
                              [list(sa.ap[0]), [126, NGRP], [1, W]])
                nc.vector.tensor_tensor(se[:, :], in0, in1, OP.add)
                anw_state[si] = se

            def anw_shf(si, tw=None):
                """Channel 4x replication via selector matmuls: psum cols
                (r, g, t); Act copy reorders to (g, r, t) = (b, t)."""
                t0, W = tw if tw is not None else _win(si)
                se = anw_state.pop(si)
                shf = pss.tile([128, 4 * NGRP * W], dt.float32, tag="win",
                               bufs=1, name="shf")
                for r in range(4):
                    nc.tensor.matmul(shf[:, r * NGRP * W:(r + 1) * NGRP * W],
                                     selt[r][:, :], se[:, :],
                                     start=True, stop=True)
                e4 = e4p.tile([128, 4 * NGRP * W], dt.float32, tag="e4",
                              name="e4")
                sh = shf[:, :]
                dst = bass.AP(e4.tensor, e4.offset,
                              [list(e4.ap[0]), [W, 4], [4 * W, NGRP], [1, W]])
                nc.scalar.activation(dst, sh, AF.Copy)
                anw_state[si] = e4

            def anw_an(si, tw=None):
                """AN spikes + bushy currents, cols (b, t) -> Qb columns."""
                t0, W = tw if tw is not None else _win(si)
                e4 = anw_state.pop(si)
                # fp16 hi/lo split of W_bushy: spikes (0/1) and 2^-11 are
                # fp16-exact, so cur_b = Wbh@spk + Wbl_s@(2^-11 spk) matches
                # fp32 up to a 2^-22 residual (verified: 0 spike flips)
                ps_cb = pss.tile([HID, 4 * NGRP * W], dt.float32, tag="win",
                                 bufs=1, name="ps_cb")
                for ch in range(3):
                    an = anp.tile([128, 4 * NGRP * W], dt.float16, tag="an",
                                  name="an")
                    nc.vector.tensor_scalar(an[:, :], e4[:, :],
                                            svt[:, ch:ch + 1], AN_THR,
                                            OP.mult, OP.is_gt)
                    an2 = anp.tile([128, 4 * NGRP * W], dt.float16, tag="an2",
                                   name="an2")
                    nc.vector.tensor_scalar(an2[:, :], an[:, :],
                                            1.0 / 2048.0, None, OP.mult)
                    nc.tensor.matmul(ps_cb[:, :], wbt[2 * ch][:, :], an[:, :],
                                     start=(ch == 0), stop=False)
                    nc.tensor.matmul(ps_cb[:, :], wbt[2 * ch + 1][:, :],
                                     an2[:, :],
                                     start=False, stop=(ch == 2))
                qap = Qb[:, :]
                dest = bass.AP(qap.tensor, qap.offset + t0 + 1,
                               [list(qap.ap[0]), [BST, BLOC], [1, W]])
                nc.scalar.activation(dest, ps_cb[:, :], AF.Copy)

            def tsl(ap2d, tau):
                # strided time-slice: columns b*BST + tau for b in 0..BLOC
                return bass.AP(ap2d.tensor, ap2d.offset + tau,
                               [list(ap2d.ap[0]), [BST, BLOC]])

            def wavefront():
                """Generator: packed LIF wavefront (32 rows), yields per step."""
                m_all, s_all = Mt[:, :], St[:, :]
                m_lo, m_hi = Mt[0:HID, :], Mt[PIC:PTOT, :]
                s_mm = St[0:PIC + HID, :]
                q_all = Qb[:, :]
                qia = {}
                for tau in range(1, NSTEP + 1):
                    nc.vector.scalar_tensor_tensor(
                        tsl(m_lo, tau), tsl(m_lo, tau - 1), BETA,
                        tsl(q_all, tau), OP.mult, OP.add)
                    if tau <= DELTA:
                        nc.vector.tensor_scalar(tsl(m_hi, tau),
                                                tsl(m_hi, tau - 1), BETA,
                                                None, OP.mult)
                    else:
                        qt = qia.pop(tau)
                        nc.vector.scalar_tensor_tensor(
                            tsl(m_hi, tau), tsl(m_hi, tau - 1), BETA,
                            qt[PIC:PTOT, :], OP.mult, OP.add)
                    nc.vector.tensor_scalar(tsl(s_all, tau), tsl(m_all, tau),
                                            THR, None, OP.is_gt)
                    if tau + DELTA <= NSTEP:
                        ps = psn.tile([PTOT, BLOC], dt.float32, tag="snn",
                                      bufs=2, name="ps_snn")
                        nc.tensor.matmul(ps[PIC:PTOT, :], wcat[:, :],
                                         tsl(s_mm, tau),
                                         start=True, stop=True)
                        qt = qip.tile([PTOT, BLOC], dt.float32, tag="qia",
                                      name="qia")
                        qia[tau + DELTA] = qt
                        nc.scalar.activation(qt[PIC:PTOT, :], ps[PIC:PTOT, :],
                                             AF.Copy)
                    nc.vector.tensor_tensor(tsl(m_all, tau), tsl(m_all, tau),
                                            tsl(s_all, tau), OP.subtract)
                    yield

            wf = wavefront()
            emitted = [0]

            def pump(upto):
                upto = min(upto, NSTEP)
                while emitted[0] < upto:
                    next(wf)
                    emitted[0] += 1

            # an_window for strip si-1 is pipelined across the strips of
            # si (se add at g==1, selector matmuls at g==2, AN at g==3) so
            # the PE never waits on the cross-engine envelope chain.
            for si in range(NSTRIP - 1):
                for g in range(NGRP):
                    if si == 0:
                        # narrow pieces: the PE can start after ~0.8us of
                        # DMA instead of 3us, shortening the p-state ramp
                        if g == 0:
                            for p4 in range(4):
                                conv_strip(g, 0, piece=(512 * p4, 512))
                            load_aux_weights()
                        else:
                            conv_strip(g, 0, piece=(0, 1024))
                            conv_strip(g, 0, piece=(1024, 1024))
                        continue
                    conv_strip(g, si)
                    if si >= 1:
                        if g == 1:
                            anw_se(si - 1)
                        elif g == 2:
                            anw_shf(si - 1)
                        elif g == 3:
                            anw_an(si - 1)
                    # pace wavefront steps unlocked by window si-2 early in
                    # this strip row, window si-1 once anw_an(si-1) ran
                    if g < 4:
                        pump(16 * si - 33 + 4 * (g + 1))
                    else:
                        pump(16 * si - 17 + 4 * (g - 3))
            # strip 7 runs as two piece passes across all groups so the
            # final window's first chunk (t 111..117, needing sample blocks
            # <= 118) overlaps the second piece's conv
            for g in range(NGRP):
                conv_strip(g, 7, piece=(14336, 1024))
                if g == 1:
                    anw_se(6)
                elif g == 2:
                    anw_shf(6)
                elif g == 3:
                    anw_an(6)
                # window 6 currents land at anw_an(6) (g==3); window 5
                # allows tau <= 95 before that
                pump(95 if g < 4 else 95 + 4 * (g - 3))
            anw_se((7, 0), (111, 8))
            anw_shf((7, 0), (111, 8))
            anw_an((7, 0), (111, 8))
            for g in range(NGRP):
                conv_strip(g, 7, piece=(15360, 640))
                pump(min(119, 112 + (g + 1)))
            anw_se((7, 1), (119, 5))
            anw_shf((7, 1), (119, 5))
            anw_an((7, 1), (119, 5))
            pump(NSTEP)

            nc.sync.dma_start(out=ospk[:, :], in_=St[PIC + HID:PTOT, :])
            nc.sync.dma_start(out=omem[:, :], in_=Mt[PIC + HID:PTOT, :])

            psn.release()
            pss.release()
            psp.release()
            qip.release()
            e4p.release()
            sep.release()
            anp.release()
            ybp.release()
            hkp.release()

    nc.finalize()
    return nc


def _prep_inputs(audio, gt_kernels, W_bushy, W_ic, W_ac):
    audio = np.ascontiguousarray(audio, dtype=np.float32)
    gt = np.ascontiguousarray(gt_kernels, dtype=np.float32)
    Wb = np.ascontiguousarray(W_bushy, dtype=np.float32)

    gth = gt.astype(np.float16)
    gtl = ((gt - gth.astype(np.float32)) * 2048.0).astype(np.float16)
    lw = np.zeros((6, 128, 128), np.float16)
    for r in range(4):
        sl = slice(r * 32, r * 32 + 32)
        # lhsT[r*32+k, r*32+c] = gt[c, k]; order: {2^11 kh, 2^11 kl, kh}
        # per tap half
        lw[0, sl, sl] = (gth[:, 0:32].astype(np.float32).T * 2048.0
                         ).astype(np.float16)
        lw[1, sl, sl] = gtl[:, 0:32].T
        lw[2, sl, sl] = gth[:, 0:32].T
        lw[3, sl, sl] = (gth[:, 32:64].astype(np.float32).T * 2048.0
                         ).astype(np.float16)
        lw[4, sl, sl] = gtl[:, 32:64].T
        lw[5, sl, sl] = gth[:, 32:64].T

    wb32 = np.zeros((3, 128, HID), np.float32)
    sv = np.zeros((128, 3), np.float32)
    for ch in range(3):
        for u in range(4):
            a = ch * 4 + u
            if a >= ANS:
                continue
            # wb32[ch, u*32+c, h] = W_bushy[h, c*10 + a]
            wb32[ch, u * 32:u * 32 + 32, :] = Wb[:, a::ANS].T
            # AN consumes raw 2^11-scaled two-block sums: fold the /256
            # and the conv 2^-11 into the scale (exact powers of two)
            sv[u * 32:u * 32 + 32, ch] = _SCALES[a] / (256.0 * 2048.0)
    wb = np.zeros((6, 128, HID), np.float16)
    for ch in range(3):
        wbh = wb32[ch].astype(np.float16)
        wb[2 * ch] = wbh
        wb[2 * ch + 1] = ((wb32[ch] - wbh.astype(np.float32)) * 2048.0
                          ).astype(np.float16)
    selr = np.zeros((4, 128, 128), np.float32)
    for r in range(4):
        for u in range(4):
            for c in range(32):
                selr[r, r * 32 + c, u * 32 + c] = 1.0
    # combined IC/AC lhsT: out partitions 0-49 = IC currents from spk_b
    # (spikes at partitions 0-49), 50-59 = AC currents from spk_ic (spikes
    # at partitions 64-113); contraction rows 50-63 are zero
    wca = np.zeros((PIC + HID, HID + OUT), np.float32)
    wca[0:HID, 0:HID] = np.ascontiguousarray(W_ic.T, dtype=np.float32)
    wca[PIC:PIC + HID, HID:HID + OUT] = np.ascontiguousarray(W_ac.T,
                                                             dtype=np.float32)

    apad = np.zeros((B, NPAD), np.float32)
    apad[:, PAD_L:PAD_L + N] = audio
    apadh = apad.astype(np.float16)
    apadl = ((apad - apadh.astype(np.float32)) * 2048.0).astype(np.float16)

    in_maps = []
    for c in range(NCORES):
        rows = slice(c * BLOC, (c + 1) * BLOC)
        in_maps.append({"apadh": apadh[rows], "apadl": apadl[rows],
                        "lw": lw, "wb": wb,
                        "wca": wca, "sv": sv, "selr": selr})
    return in_maps


def kernel(audio, gt_kernels, W_bushy, W_ic, W_ac, _trace=False):
    global _NC_CACHE
    if _NC_CACHE is None:
        _NC_CACHE = _build_nc()
    nc = _NC_CACHE
    in_maps = _prep_inputs(audio, gt_kernels, W_bushy, W_ic, W_ac)
    res = run_bass_kernel_spmd(nc, in_maps, core_ids=list(range(NCORES)),
                               trace=_trace)
    spk = np.empty((B, T, OUT), np.float32)
    mem = np.empty((B, T, OUT), np.float32)
    for c in range(NCORES):
        rs = res.results[c]["ospk"].reshape(OUT, BLOC, BST)
        rm = res.results[c]["omem"].reshape(OUT, BLOC, BST)
        rows = slice(c * BLOC, (c + 1) * BLOC)
        # [o, b, tau] -> [b, t, o] with t = tau - OFF0
        spk[rows] = rs[:, :, OFF0:OFF0 + T].transpose(1, 2, 0)
        mem[rows] = rm[:, :, OFF0:OFF0 + T].transpose(1, 2, 0)
    kernel._last_results = res
    return spk, mem


# revision 32
# speedup vs baseline: 1.0183x; 1.0048x over previous
"""Trainium2 Bass kernel for the gammatone-cochlea + LIF-SNN model.

Pipeline per core (32 of the 256 batch rows, pure data parallel):
  1. Gammatone conv [32ch, 64 taps] via tap-split Hankel matmuls (fp32 PE):
     4 batch rows per 128-partition group, block-diagonal lhsT, two
     accumulating matmuls per 512-sample block (taps 0-31 / 32-63, the
     second reading the same Hankel tile at free offset +32). One combined
     DMA per strip loads the 128-partition Hankel. The strip loop is
     TIME-MAJOR (strip index outer, group inner) so that after strip si
     every batch row's envelope is known for t < 16*si + 15.
  2. ReLU on ScalarE (PSUM -> SBUF copy), then DVE strided block-sums
     (128-sample blocks) into S_all. The /256 framing normalization is
     folded into the AN scales.
  3. Per strip: one strided DVE add forms the two-block sums for the new
     t-window across all 8 groups, selector matmuls replicate channels
     4x across partitions (u,c), and the AN stage (fused tensor_scalar
     mult+is_gt, 3 accumulating matmuls) produces bushy currents which
     land in Q[0:50] at columns b*136 + 1 + t.
  4. SNN: one 32-row packed wavefront. All three LIF layers (bushy 50,
     IC at partitions 64-113, AC at 114-123) update in [124,32] DVE ops:
     scalar_tensor_tensor (beta*mem + cur), is_gt spike, subtract reset.
     IC lags bushy by DELTA steps and AC by 2*DELTA; the per-step PE
     matmul blockdiag(WicT,WacT) @ [spk_b; spk_ic] and its ScalarE
     PSUM->SBUF copy have DELTA steps of slack. Wavefront steps are
     paced through the conv loop, so only ~23 steps remain as tail.
Outputs: spikes/membranes live in partitions 114-123 of the state tiles,
columns b*136 + t + 1 + 2*DELTA; host slices + transposes.
"""
import numpy as np
import concourse.bass as bass
import concourse.bacc as bacc
import concourse.mybir as mybir
import concourse.tile as tile
from concourse.bass_utils import run_bass_kernel_spmd

dt = mybir.dt
AF = mybir.ActivationFunctionType
OP = mybir.AluOpType

NCORES = 8
B, N, C, K = 256, 16000, 32, 64
BLOC = B // NCORES            # 32 batch rows per core
WINDOW, STRIDE, T = 256, 128, 124
ANS, HID, OUT = 10, 50, 10
BETA, THR, AN_THR = 0.95, 1.0, 0.5
PAD_L, PAD_R = 31, 33         # SAME padding for K=64: 31 left, 32 right (+1 slack)
NPAD = PAD_L + N + PAD_R      # 16064
NGRP = BLOC // 4              # 8 groups of 4 rows
NSTRIP = 8
STRIPS = [2048] * 7 + [1664]  # 4-block strips per group

# packed-wavefront SNN layout (partition-sliced ops must start at a
# quadrant boundary, so IC sits at partitions 64-113, AC at 114-123, and
# partitions 50-63 stay zero)
DELTA = 2                     # IC lags bushy by DELTA steps, AC by 2*DELTA
PIC, PTOT = 64, 124           # IC partition base; total SNN partitions
BST = 132                     # per-batch-row column stride (1 + 124 + 2*DELTA <= BST)
FREE = BLOC * BST             # 4352
NSTEP = T + 1 + 2 * DELTA     # wavefront steps tau = 1..NSTEP (130)
OFF0 = 1 + 2 * DELTA          # AC output for t sits at column b*BST + OFF0 + t

# jnp.linspace(0.5, 1.5, 10, dtype=f32), bitexact
_SCALES = np.array([0x3F000000, 0x3F1C71C7, 0x3F38E38E, 0x3F555555, 0x3F71C71D,
                    0x3F871C72, 0x3F955556, 0x3FA38E39, 0x3FB1C71D, 0x3FC00000],
                   dtype=np.uint32).view(np.float32)

_NC_CACHE = None


def _build_nc():
    nc = bacc.Bacc("TRN2", target_bir_lowering=False, debug=False,
                   num_devices=NCORES)

    apadh = nc.dram_tensor("apadh", [BLOC, NPAD], dt.float16,
                           kind="ExternalInput")
    apadl = nc.dram_tensor("apadl", [BLOC, NPAD], dt.float16,
                           kind="ExternalInput")
    lw = nc.dram_tensor("lw", [6, 128, 128], dt.float16, kind="ExternalInput")
    wb = nc.dram_tensor("wb", [6, 128, HID], dt.float16, kind="ExternalInput")
    wca = nc.dram_tensor("wca", [PIC + HID, HID + OUT], dt.float32,
                         kind="ExternalInput")
    sv = nc.dram_tensor("sv", [128, 3], dt.float32, kind="ExternalInput")
    selr = nc.dram_tensor("selr", [4, 128, 128], dt.float32, kind="ExternalInput")
    ospk = nc.dram_tensor("ospk", [OUT, FREE], dt.float32, kind="ExternalOutput")
    omem = nc.dram_tensor("omem", [OUT, FREE], dt.float32, kind="ExternalOutput")

    with tile.TileContext(nc) as tc:
        with tc.tile_pool(name="cpool", bufs=1) as cp:
            # stationary conv operands (fp16): per tap-half {2^11*kh,
            # 2^11*kl, kh}; paired with moving {ah, ah, 2^11*al}
            lwt = [cp.tile([128, 128], dt.float16, name=f"lw{i}")
                   for i in range(6)]
            for i in range(6):
                nc.gpsimd.dma_start(out=lwt[i][:, :], in_=lw[i, :, :])
            svt = cp.tile([128, 3], dt.float32)
            wbt = [cp.tile([128, HID], dt.float16, name=f"wbt{i}") for i in range(6)]
            wcat = cp.tile([PIC + HID, HID + OUT], dt.float32)
            selt = [cp.tile([128, 128], dt.float32, name=f"selt{r}")
                    for r in range(4)]

            def load_aux_weights():
                # deferred until after si=0's Hankel DMAs: none of these are
                # consumed before the si=1 an_window / wavefront stages
                nc.gpsimd.dma_start(out=svt[:, :], in_=sv[:, :])
                for i in range(6):
                    nc.gpsimd.dma_start(out=wbt[i][:, :], in_=wb[i, :, :])
                nc.gpsimd.dma_start(out=wcat[:, :], in_=wca[:, :])
                for r in range(4):
                    nc.gpsimd.dma_start(out=selt[r][:, :], in_=selr[r, :, :])

            S_all = cp.tile([128, NGRP * 126], dt.float32)

            # packed SNN state (32 rows wide)
            Mt = cp.tile([PTOT, FREE], dt.float32, name="Mt")
            St = cp.tile([PTOT, FREE], dt.float32, name="St")
            Qb = cp.tile([HID, FREE], dt.float32, name="Qb")
            nc.gpsimd.memset(Mt[:, :], 0.0)
            nc.gpsimd.memset(St[:, :], 0.0)
            nc.gpsimd.memset(Qb[:, :], 0.0)

            hkp = tc.alloc_tile_pool(name="hkp", bufs=8)
            ybp = tc.alloc_tile_pool(name="ybp", bufs=8)
            anp = tc.alloc_tile_pool(name="anp", bufs=2)
            sep = tc.alloc_tile_pool(name="sep", bufs=2)
            e4p = tc.alloc_tile_pool(name="e4p", bufs=2)
            qip = tc.alloc_tile_pool(name="qip", bufs=DELTA + 3)
            psp = tc.alloc_tile_pool(name="psp", bufs=1, space="PSUM")
            pss = tc.alloc_tile_pool(name="pss", bufs=1, space="PSUM")
            psn = tc.alloc_tile_pool(name="psn", bufs=1, space="PSUM")

            # PE clock warm-up: one throwaway matmul as soon as the first
            # stationary tile lands (~0.4us) starts the HAM p-state ramp,
            # so conv matmuls reach full clock ~2us earlier
            wps = pss.tile([128, 128], dt.float32, tag="win", bufs=1,
                           name="warm")
            nc.tensor.matmul(wps[:, :], lwt[0][:, :], lwt[0][:, :],
                             start=True, stop=True)

            strip_ctr = [0]

            def conv_strip(g, si, piece=None):
                """Conv + framing for rows 4g..4g+4, samples 2048si..+sw."""
                sc = strip_ctr[0]
                strip_ctr[0] += 1
                if piece is None:
                    sw = STRIPS[si]
                    s0 = 2048 * si
                else:
                    s0, sw = piece
                hkh = hkp.tile([128, 2112], dt.float16, tag="hkh", name="hkh")
                hkl = hkp.tile([128, 2112], dt.float16, tag="hkl", name="hkl")
                # Hankel: hk[32*r+k, j] = apad[4g+r, s0 + j + k], one DMA each
                srch = bass.AP(apadh, (4 * g) * NPAD + s0,
                               [[NPAD, 4], [1, 32], [1, sw + 32]])
                nc.sync.dma_start(out=hkh[:, 0:sw + 32], in_=srch)
                srcl = bass.AP(apadl, (4 * g) * NPAD + s0,
                               [[NPAD, 4], [1, 32], [1, sw + 32]])
                nc.sync.dma_start(out=hkl[:, 0:sw + 32], in_=srcl)
                nb4 = (sw + 511) // 512
                accs = []
                # fp16 hi/lo split, 2^11-scaled PSUM: for each tap half,
                # accumulate ah*(2^11 kh) + ah*(2^11 kl) + (2^11 al)*kh;
                # the al*kl term is below fp32 rounding. The 2^-11 is
                # folded into the AN scales (relu commutes with it).
                passes = [(0, hkh, 0), (1, hkh, 0), (2, hkl, 0),
                          (3, hkh, 32), (4, hkh, 32), (5, hkl, 32)]
                for b4 in range(nb4):
                    w = min(512, sw - 512 * b4)
                    acc = psp.tile([128, 512], dt.float32,
                                   tag=f"acc{(4 * sc + b4) % 5}", name="acc")
                    accs.append((acc, w))
                for pi, (li, hkt, off) in enumerate(passes):
                    for b4 in range(nb4):
                        acc, w = accs[b4]
                        nc.tensor.matmul(
                            acc[:, 0:w], lwt[li][:, :],
                            hkt[:, 512 * b4 + off:512 * b4 + off + w],
                            start=(pi == 0), stop=(pi == 5))
                for b4 in range(nb4):
                    acc, w = accs[b4]
                    yb = ybp.tile([128, 512], dt.float32, tag="yb", name="yb")
                    nc.scalar.activation(yb[:, 0:w], acc[:, 0:w], AF.Relu)
                    nblk = w // 128
                    i = s0 // 512 + b4
                    view = bass.AP(yb.tensor, yb.offset,
                                   [list(yb.ap[0]), [128, nblk], [1, 128]])
                    nc.vector.tensor_reduce(
                        S_all[:, g * 126 + 4 * i: g * 126 + 4 * i + nblk],
                        view, axis=mybir.AxisListType.X, op=OP.add)

            def _win(si):
                t0 = max(0, 16 * si - 1)
                t1 = min(T - 1, 16 * si + 14)
                return t0, t1 - t0 + 1

            anw_state = {}

            def anw_se(si, tw=None):
                """Two-block sums for the t-window unlocked by strip si:
                t in [max(0,16si-1), min(123,16si+14)], cols (g, t)."""
                t0, W = tw if tw is not None else _win(si)
                se = sep.tile([128, NGRP * W], dt.float32, tag="senv",
                              name="senv")
                sa = S_all[:, :]
                in0 = bass.AP(sa.tensor, sa.offset + t0,
                              [list(sa.ap[0]), [126, NGRP], [1, W]])
                in1 = bass.AP(sa.tensor, sa.offset + t0 + 1,
                              [list(sa.ap[0]), [126, NGRP], [1, W]])
                nc.vector.tensor_tensor(se[:, :], in0, in1, OP.add)
                anw_state[si] = se

            def anw_shf(si, tw=None):
                """Channel 4x replication via selector matmuls: psum cols
                (r, g, t); Act copy reorders to (g, r, t) = (b, t)."""
                t0, W = tw if tw is not None else _win(si)
                se = anw_state.pop(si)
                shf = pss.tile([128, 4 * NGRP * W], dt.float32, tag="win",
                               bufs=1, name="shf")
                for r in range(4):
                    nc.tensor.matmul(shf[:, r * NGRP * W:(r + 1) * NGRP * W],
                                     selt[r][:, :], se[:, :],
                                     start=True, stop=True)
                e4 = e4p.tile([128, 4 * NGRP * W], dt.float32, tag="e4",
                              name="e4")
                sh = shf[:, :]
                dst = bass.AP(e4.tensor, e4.offset,
                              [list(e4.ap[0]), [W, 4], [4 * W, NGRP], [1, W]])
                nc.scalar.activation(dst, sh, AF.Copy)
                anw_state[si] = e4

            def anw_an(si, tw=None):
                """AN spikes + bushy currents, cols (b, t) -> Qb columns."""
                t0, W = tw if tw is not None else _win(si)
                e4 = anw_state.pop(si)
                # fp16 hi/lo split of W_bushy: spikes (0/1) and 2^-11 are
                # fp16-exact, so cur_b = Wbh@spk + Wbl_s@(2^-11 spk) matches
                # fp32 up to a 2^-22 residual (verified: 0 spike flips)
                ps_cb = pss.tile([HID, 4 * NGRP * W], dt.float32, tag="win",
                                 bufs=1, name="ps_cb")
                for ch in range(3):
                    an = anp.tile([128, 4 * NGRP * W], dt.float16, tag="an",
                                  name="an")
                    nc.vector.tensor_scalar(an[:, :], e4[:, :],
                                            svt[:, ch:ch + 1], AN_THR,
                                            OP.mult, OP.is_gt)
                    an2 = anp.tile([128, 4 * NGRP * W], dt.float16, tag="an2",
                                   name="an2")
                    nc.vector.tensor_scalar(an2[:, :], an[:, :],
                                            1.0 / 2048.0, None, OP.mult)
                    nc.tensor.matmul(ps_cb[:, :], wbt[2 * ch][:, :], an[:, :],
                                     start=(ch == 0), stop=False)
                    nc.tensor.matmul(ps_cb[:, :], wbt[2 * ch + 1][:, :],
                                     an2[:, :],
                                     start=False, stop=(ch == 2))
                qap = Qb[:, :]
                dest = bass.AP(qap.tensor, qap.offset + t0 + 1,
                               [list(qap.ap[0]), [BST, BLOC], [1, W]])
                nc.scalar.activation(dest, ps_cb[:, :], AF.Copy)

            def tsl(ap2d, tau):
                # strided time-slice: columns b*BST + tau for b in 0..BLOC
                return bass.AP(ap2d.tensor, ap2d.offset + tau,
                               [list(ap2d.ap[0]), [BST, BLOC]])

            def wavefront():
                """Generator: packed LIF wavefront (32 rows), yields per step."""
                m_all, s_all = Mt[:, :], St[:, :]
                m_lo, m_hi = Mt[0:HID, :], Mt[PIC:PTOT, :]
                s_mm = St[0:PIC + HID, :]
                q_all = Qb[:, :]
                qia = {}
                for tau in range(1, NSTEP + 1):
                    nc.vector.scalar_tensor_tensor(
                        tsl(m_lo, tau), tsl(m_lo, tau - 1), BETA,
                        tsl(q_all, tau), OP.mult, OP.add)
                    if tau <= DELTA:
                        nc.vector.tensor_scalar(tsl(m_hi, tau),
                                                tsl(m_hi, tau - 1), BETA,
                                                None, OP.mult)
                    else:
                        qt = qia.pop(tau)
                        nc.vector.scalar_tensor_tensor(
                            tsl(m_hi, tau), tsl(m_hi, tau - 1), BETA,
                            qt[PIC:PTOT, :], OP.mult, OP.add)
                    nc.vector.tensor_scalar(tsl(s_all, tau), tsl(m_all, tau),
                                            THR, None, OP.is_gt)
                    if tau + DELTA <= NSTEP:
                        ps = psn.tile([PTOT, BLOC], dt.float32, tag="snn",
                                      bufs=2, name="ps_snn")
                        nc.tensor.matmul(ps[PIC:PTOT, :], wcat[:, :],
                                         tsl(s_mm, tau),
                                         start=True, stop=True)
                        qt = qip.tile([PTOT, BLOC], dt.float32, tag="qia",
                                      name="qia")
                        qia[tau + DELTA] = qt
                        nc.scalar.activation(qt[PIC:PTOT, :], ps[PIC:PTOT, :],
                                             AF.Copy)
                    nc.vector.tensor_tensor(tsl(m_all, tau), tsl(m_all, tau),
                                            tsl(s_all, tau), OP.subtract)
                    yield

            wf = wavefront()
            emitted = [0]

            def pump(upto):
                upto = min(upto, NSTEP)
                while emitted[0] < upto:
                    next(wf)
                    emitted[0] += 1

            # an_window for strip si-1 is pipelined across the strips of
            # si (se add at g==1, selector matmuls at g==2, AN at g==3) so
            # the PE never waits on the cross-engine envelope chain.
            for si in range(NSTRIP - 1):
                for g in range(NGRP):
                    if si == 0:
                        # narrow pieces: the PE can start after ~0.8us of
                        # DMA instead of 3us, shortening the p-state ramp
                        if g == 0:
                            for p4 in range(4):
                                conv_strip(g, 0, piece=(512 * p4, 512))
                            load_aux_weights()
                        else:
                            conv_strip(g, 0, piece=(0, 1024))
                            conv_strip(g, 0, piece=(1024, 1024))
                        continue
                    conv_strip(g, si)
                    if si >= 1:
                        if g == 1:
                            anw_se(si - 1)
                        elif g == 2:
                            anw_shf(si - 1)
                        elif g == 3:
                            anw_an(si - 1)
                    # pace wavefront steps unlocked by window si-2 early in
                    # this strip row, window si-1 once anw_an(si-1) ran
                    if g < 4:
                        pump(16 * si - 33 + 4 * (g + 1))
                    else:
                        pump(16 * si - 17 + 4 * (g - 3))
            # strip 7 runs as two piece passes across all groups so the
            # final window's first chunk (t 111..117, needing sample blocks
            # <= 118) overlaps the second piece's conv
            for g in range(NGRP):
                conv_strip(g, 7, piece=(14336, 1024))
                if g == 1:
                    anw_se(6)
                elif g == 2:
                    anw_shf(6)
                elif g == 3:
                    anw_an(6)
                # window 6 currents land at anw_an(6) (g==3); window 5
                # allows tau <= 95 before that
                pump(95 if g < 4 else 95 + 4 * (g - 3))
            anw_se((7, 0), (111, 8))
            anw_shf((7, 0), (111, 8))
            anw_an((7, 0), (111, 8))
            for g in range(NGRP):
                conv_strip(g, 7, piece=(15360, 512))
                pump(min(119, 112 + (g + 1)))
            anw_se((7, 1), (119, 4))
            anw_shf((7, 1), (119, 4))
            anw_an((7, 1), (119, 4))
            for g in range(NGRP):
                conv_strip(g, 7, piece=(15872, 128))
                pump(min(123, 119 + (g + 1)))
            anw_se((7, 2), (123, 1))
            anw_shf((7, 2), (123, 1))
            anw_an((7, 2), (123, 1))
            pump(NSTEP)

            nc.sync.dma_start(out=ospk[:, :], in_=St[PIC + HID:PTOT, :])
            nc.sync.dma_start(out=omem[:, :], in_=Mt[PIC + HID:PTOT, :])

            psn.release()
            pss.release()
            psp.release()
            qip.release()
            e4p.release()
            sep.release()
            anp.release()
            ybp.release()
            hkp.release()

    nc.finalize()
    return nc


def _prep_inputs(audio, gt_kernels, W_bushy, W_ic, W_ac):
    audio = np.ascontiguousarray(audio, dtype=np.float32)
    gt = np.ascontiguousarray(gt_kernels, dtype=np.float32)
    Wb = np.ascontiguousarray(W_bushy, dtype=np.float32)

    gth = gt.astype(np.float16)
    gtl = ((gt - gth.astype(np.float32)) * 2048.0).astype(np.float16)
    lw = np.zeros((6, 128, 128), np.float16)
    for r in range(4):
        sl = slice(r * 32, r * 32 + 32)
        # lhsT[r*32+k, r*32+c] = gt[c, k]; order: {2^11 kh, 2^11 kl, kh}
        # per tap half
        lw[0, sl, sl] = (gth[:, 0:32].astype(np.float32).T * 2048.0
                         ).astype(np.float16)
        lw[1, sl, sl] = gtl[:, 0:32].T
        lw[2, sl, sl] = gth[:, 0:32].T
        lw[3, sl, sl] = (gth[:, 32:64].astype(np.float32).T * 2048.0
                         ).astype(np.float16)
        lw[4, sl, sl] = gtl[:, 32:64].T
        lw[5, sl, sl] = gth[:, 32:64].T

    wb32 = np.zeros((3, 128, HID), np.float32)
    sv = np.zeros((128, 3), np.float32)
    for ch in range(3):
        for u in range(4):
            a = ch * 4 + u
            if a >= ANS:
                continue
            # wb32[ch, u*32+c, h] = W_bushy[h, c*10 + a]
            wb32[ch, u * 32:u * 32 + 32, :] = Wb[:, a::ANS].T
            # AN consumes raw 2^11-scaled two-block sums: fold the /256
            # and the conv 2^-11 into the scale (exact powers of two)
            sv[u * 32:u * 32 + 32, ch] = _SCALES[a] / (256.0 * 2048.0)
    wb = np.zeros((6, 128, HID), np.float16)
    for ch in range(3):
        wbh = wb32[ch].astype(np.float16)
        wb[2 * ch] = wbh
        wb[2 * ch + 1] = ((wb32[ch] - wbh.astype(np.float32)) * 2048.0
                          ).astype(np.float16)
    selr = np.zeros((4, 128, 128), np.float32)
    for r in range(4):
        for u in range(4):
            for c in range(32):
                selr[r, r * 32 + c, u * 32 + c] = 1.0
    # combined IC/AC lhsT: out partitions 0-49 = IC currents from spk_b
    # (spikes at partitions 0-49), 50-59 = AC currents from spk_ic (spikes
    # at partitions 64-113); contraction rows 50-63 are zero
    wca = np.zeros((PIC + HID, HID + OUT), np.float32)
    wca[0:HID, 0:HID] = np.ascontiguousarray(W_ic.T, dtype=np.float32)
    wca[PIC:PIC + HID, HID:HID + OUT] = np.ascontiguousarray(W_ac.T,
                                                             dtype=np.float32)

    apad = np.zeros((B, NPAD), np.float32)
    apad[:, PAD_L:PAD_L + N] = audio
    apadh = apad.astype(np.float16)
    apadl = ((apad - apadh.astype(np.float32)) * 2048.0).astype(np.float16)

    in_maps = []
    for c in range(NCORES):
        rows = slice(c * BLOC, (c + 1) * BLOC)
        in_maps.append({"apadh": apadh[rows], "apadl": apadl[rows],
                        "lw": lw, "wb": wb,
                        "wca": wca, "sv": sv, "selr": selr})
    return in_maps


def kernel(audio, gt_kernels, W_bushy, W_ic, W_ac, _trace=False):
    global _NC_CACHE
    if _NC_CACHE is None:
        _NC_CACHE = _build_nc()
    nc = _NC_CACHE
    in_maps = _prep_inputs(audio, gt_kernels, W_bushy, W_ic, W_ac)
    res = run_bass_kernel_spmd(nc, in_maps, core_ids=list(range(NCORES)),
                               trace=_trace)
    spk = np.empty((B, T, OUT), np.float32)
    mem = np.empty((B, T, OUT), np.float32)
    for c in range(NCORES):
        rs = res.results[c]["ospk"].reshape(OUT, BLOC, BST)
        rm = res.results[c]["omem"].reshape(OUT, BLOC, BST)
        rows = slice(c * BLOC, (c + 1) * BLOC)
        # [o, b, tau] -> [b, t, o] with t = tau - OFF0
        spk[rows] = rs[:, :, OFF0:OFF0 + T].transpose(1, 2, 0)
        mem[rows] = rm[:, :, OFF0:OFF0 + T].transpose(1, 2, 0)
    kernel._last_results = res
    return spk, mem


# revision 33
# speedup vs baseline: 1.0204x; 1.0021x over previous
"""Trainium2 Bass kernel for the gammatone-cochlea + LIF-SNN model.

Pipeline per core (32 of the 256 batch rows, pure data parallel):
  1. Gammatone conv [32ch, 64 taps] via tap-split Hankel matmuls (fp32 PE):
     4 batch rows per 128-partition group, block-diagonal lhsT, two
     accumulating matmuls per 512-sample block (taps 0-31 / 32-63, the
     second reading the same Hankel tile at free offset +32). One combined
     DMA per strip loads the 128-partition Hankel. The strip loop is
     TIME-MAJOR (strip index outer, group inner) so that after strip si
     every batch row's envelope is known for t < 16*si + 15.
  2. ReLU on ScalarE (PSUM -> SBUF copy), then DVE strided block-sums
     (128-sample blocks) into S_all. The /256 framing normalization is
     folded into the AN scales.
  3. Per strip: one strided DVE add forms the two-block sums for the new
     t-window across all 8 groups, selector matmuls replicate channels
     4x across partitions (u,c), and the AN stage (fused tensor_scalar
     mult+is_gt, 3 accumulating matmuls) produces bushy currents which
     land in Q[0:50] at columns b*136 + 1 + t.
  4. SNN: one 32-row packed wavefront. All three LIF layers (bushy 50,
     IC at partitions 64-113, AC at 114-123) update in [124,32] DVE ops:
     scalar_tensor_tensor (beta*mem + cur), is_gt spike, subtract reset.
     IC lags bushy by DELTA steps and AC by 2*DELTA; the per-step PE
     matmul blockdiag(WicT,WacT) @ [spk_b; spk_ic] and its ScalarE
     PSUM->SBUF copy have DELTA steps of slack. Wavefront steps are
     paced through the conv loop, so only ~23 steps remain as tail.
Outputs: spikes/membranes live in partitions 114-123 of the state tiles,
columns b*136 + t + 1 + 2*DELTA; host slices + transposes.
"""
import numpy as np
import concourse.bass as bass
import concourse.bacc as bacc
import concourse.mybir as mybir
import concourse.tile as tile
from concourse.bass_utils import run_bass_kernel_spmd

dt = mybir.dt
AF = mybir.ActivationFunctionType
OP = mybir.AluOpType

NCORES = 8
B, N, C, K = 256, 16000, 32, 64
BLOC = B // NCORES            # 32 batch rows per core
WINDOW, STRIDE, T = 256, 128, 124
ANS, HID, OUT = 10, 50, 10
BETA, THR, AN_THR = 0.95, 1.0, 0.5
PAD_L, PAD_R = 31, 33         # SAME padding for K=64: 31 left, 32 right (+1 slack)
NPAD = PAD_L + N + PAD_R      # 16064
NGRP = BLOC // 4              # 8 groups of 4 rows
NSTRIP = 8
STRIPS = [2048] * 7 + [1664]  # 4-block strips per group

# packed-wavefront SNN layout (partition-sliced ops must start at a
# quadrant boundary, so IC sits at partitions 64-113, AC at 114-123, and
# partitions 50-63 stay zero)
DELTA = 2                     # IC lags bushy by DELTA steps, AC by 2*DELTA
PIC, PTOT = 64, 124           # IC partition base; total SNN partitions
BST = 132                     # per-batch-row column stride (1 + 124 + 2*DELTA <= BST)
FREE = BLOC * BST             # 4352
NSTEP = T + 1 + 2 * DELTA     # wavefront steps tau = 1..NSTEP (130)
OFF0 = 1 + 2 * DELTA          # AC output for t sits at column b*BST + OFF0 + t

# jnp.linspace(0.5, 1.5, 10, dtype=f32), bitexact
_SCALES = np.array([0x3F000000, 0x3F1C71C7, 0x3F38E38E, 0x3F555555, 0x3F71C71D,
                    0x3F871C72, 0x3F955556, 0x3FA38E39, 0x3FB1C71D, 0x3FC00000],
                   dtype=np.uint32).view(np.float32)

_NC_CACHE = None


def _build_nc():
    nc = bacc.Bacc("TRN2", target_bir_lowering=False, debug=False,
                   num_devices=NCORES)

    apadh = nc.dram_tensor("apadh", [BLOC, NPAD], dt.float16,
                           kind="ExternalInput")
    apadl = nc.dram_tensor("apadl", [BLOC, NPAD], dt.float16,
                           kind="ExternalInput")
    lw = nc.dram_tensor("lw", [6, 128, 128], dt.float16, kind="ExternalInput")
    wb = nc.dram_tensor("wb", [6, 128, HID], dt.float16, kind="ExternalInput")
    wca = nc.dram_tensor("wca", [PIC + HID, HID + OUT], dt.float32,
                         kind="ExternalInput")
    sv = nc.dram_tensor("sv", [128, 3], dt.float32, kind="ExternalInput")
    selr = nc.dram_tensor("selr", [4, 128, 128], dt.float32, kind="ExternalInput")
    ospk = nc.dram_tensor("ospk", [OUT, FREE], dt.float32, kind="ExternalOutput")
    omem = nc.dram_tensor("omem", [OUT, FREE], dt.float32, kind="ExternalOutput")

    with tile.TileContext(nc) as tc:
        with tc.tile_pool(name="cpool", bufs=1) as cp:
            # stationary conv operands (fp16): per tap-half {2^11*kh,
            # 2^11*kl, kh}; paired with moving {ah, ah, 2^11*al}
            lwt = [cp.tile([128, 128], dt.float16, name=f"lw{i}")
                   for i in range(6)]
            for i in range(6):
                nc.gpsimd.dma_start(out=lwt[i][:, :], in_=lw[i, :, :])
            svt = cp.tile([128, 3], dt.float32)
            wbt = [cp.tile([128, HID], dt.float16, name=f"wbt{i}") for i in range(6)]
            wcat = cp.tile([PIC + HID, HID + OUT], dt.float32)
            selt = [cp.tile([128, 128], dt.float32, name=f"selt{r}")
                    for r in range(4)]

            def load_aux_weights():
                # deferred until after si=0's Hankel DMAs: none of these are
                # consumed before the si=1 an_window / wavefront stages
                nc.gpsimd.dma_start(out=svt[:, :], in_=sv[:, :])
                for i in range(6):
                    nc.gpsimd.dma_start(out=wbt[i][:, :], in_=wb[i, :, :])
                nc.gpsimd.dma_start(out=wcat[:, :], in_=wca[:, :])
                for r in range(4):
                    nc.gpsimd.dma_start(out=selt[r][:, :], in_=selr[r, :, :])

            S_all = cp.tile([128, NGRP * 126], dt.float32)

            # packed SNN state (32 rows wide)
            Mt = cp.tile([PTOT, FREE], dt.float32, name="Mt")
            St = cp.tile([PTOT, FREE], dt.float32, name="St")
            Qb = cp.tile([HID, FREE], dt.float32, name="Qb")
            nc.gpsimd.memset(Mt[:, :], 0.0)
            nc.gpsimd.memset(St[:, :], 0.0)
            nc.gpsimd.memset(Qb[:, :], 0.0)

            hkp = tc.alloc_tile_pool(name="hkp", bufs=8)
            ybp = tc.alloc_tile_pool(name="ybp", bufs=12)
            anp = tc.alloc_tile_pool(name="anp", bufs=2)
            sep = tc.alloc_tile_pool(name="sep", bufs=2)
            e4p = tc.alloc_tile_pool(name="e4p", bufs=2)
            qip = tc.alloc_tile_pool(name="qip", bufs=DELTA + 3)
            psp = tc.alloc_tile_pool(name="psp", bufs=1, space="PSUM")
            pss = tc.alloc_tile_pool(name="pss", bufs=1, space="PSUM")
            psn = tc.alloc_tile_pool(name="psn", bufs=1, space="PSUM")

            # PE clock warm-up: one throwaway matmul as soon as the first
            # stationary tile lands (~0.4us) starts the HAM p-state ramp,
            # so conv matmuls reach full clock ~2us earlier
            wps = pss.tile([128, 128], dt.float32, tag="win", bufs=1,
                           name="warm")
            nc.tensor.matmul(wps[:, :], lwt[0][:, :], lwt[0][:, :],
                             start=True, stop=True)

            strip_ctr = [0]

            def conv_strip(g, si, piece=None):
                """Conv + framing for rows 4g..4g+4, samples 2048si..+sw."""
                sc = strip_ctr[0]
                strip_ctr[0] += 1
                if piece is None:
                    sw = STRIPS[si]
                    s0 = 2048 * si
                else:
                    s0, sw = piece
                hkh = hkp.tile([128, 2112], dt.float16, tag="hkh", name="hkh")
                hkl = hkp.tile([128, 2112], dt.float16, tag="hkl", name="hkl")
                # Hankel: hk[32*r+k, j] = apad[4g+r, s0 + j + k], one DMA each
                srch = bass.AP(apadh, (4 * g) * NPAD + s0,
                               [[NPAD, 4], [1, 32], [1, sw + 32]])
                nc.sync.dma_start(out=hkh[:, 0:sw + 32], in_=srch)
                srcl = bass.AP(apadl, (4 * g) * NPAD + s0,
                               [[NPAD, 4], [1, 32], [1, sw + 32]])
                nc.sync.dma_start(out=hkl[:, 0:sw + 32], in_=srcl)
                nb4 = (sw + 511) // 512
                accs = []
                # fp16 hi/lo split, 2^11-scaled PSUM: for each tap half,
                # accumulate ah*(2^11 kh) + ah*(2^11 kl) + (2^11 al)*kh;
                # the al*kl term is below fp32 rounding. The 2^-11 is
                # folded into the AN scales (relu commutes with it).
                passes = [(0, hkh, 0), (1, hkh, 0), (2, hkl, 0),
                          (3, hkh, 32), (4, hkh, 32), (5, hkl, 32)]
                for b4 in range(nb4):
                    w = min(512, sw - 512 * b4)
                    acc = psp.tile([128, 512], dt.float32,
                                   tag=f"acc{(4 * sc + b4) % 5}", name="acc")
                    accs.append((acc, w))
                for pi, (li, hkt, off) in enumerate(passes):
                    for b4 in range(nb4):
                        acc, w = accs[b4]
                        nc.tensor.matmul(
                            acc[:, 0:w], lwt[li][:, :],
                            hkt[:, 512 * b4 + off:512 * b4 + off + w],
                            start=(pi == 0), stop=(pi == 5))
                for b4 in range(nb4):
                    acc, w = accs[b4]
                    yb = ybp.tile([128, 512], dt.float32, tag="yb", name="yb")
                    nc.scalar.activation(yb[:, 0:w], acc[:, 0:w], AF.Relu)
                    nblk = w // 128
                    i = s0 // 512 + b4
                    view = bass.AP(yb.tensor, yb.offset,
                                   [list(yb.ap[0]), [128, nblk], [1, 128]])
                    nc.vector.tensor_reduce(
                        S_all[:, g * 126 + 4 * i: g * 126 + 4 * i + nblk],
                        view, axis=mybir.AxisListType.X, op=OP.add)

            def _win(si):
                t0 = max(0, 16 * si - 1)
                t1 = min(T - 1, 16 * si + 14)
                return t0, t1 - t0 + 1

            anw_state = {}

            def anw_se(si, tw=None):
                """Two-block sums for the t-window unlocked by strip si:
                t in [max(0,16si-1), min(123,16si+14)], cols (g, t)."""
                t0, W = tw if tw is not None else _win(si)
                se = sep.tile([128, NGRP * W], dt.float32, tag="senv",
                              name="senv")
                sa = S_all[:, :]
                in0 = bass.AP(sa.tensor, sa.offset + t0,
                              [list(sa.ap[0]), [126, NGRP], [1, W]])
                in1 = bass.AP(sa.tensor, sa.offset + t0 + 1,
                              [list(sa.ap[0]), [126, NGRP], [1, W]])
                nc.vector.tensor_tensor(se[:, :], in0, in1, OP.add)
                anw_state[si] = se

            def anw_shf(si, tw=None):
                """Channel 4x replication via selector matmuls: psum cols
                (r, g, t); Act copy reorders to (g, r, t) = (b, t)."""
                t0, W = tw if tw is not None else _win(si)
                se = anw_state.pop(si)
                shf = pss.tile([128, 4 * NGRP * W], dt.float32, tag="win",
                               bufs=1, name="shf")
                for r in range(4):
                    nc.tensor.matmul(shf[:, r * NGRP * W:(r + 1) * NGRP * W],
                                     selt[r][:, :], se[:, :],
                                     start=True, stop=True)
                e4 = e4p.tile([128, 4 * NGRP * W], dt.float32, tag="e4",
                              name="e4")
                sh = shf[:, :]
                dst = bass.AP(e4.tensor, e4.offset,
                              [list(e4.ap[0]), [W, 4], [4 * W, NGRP], [1, W]])
                nc.scalar.activation(dst, sh, AF.Copy)
                anw_state[si] = e4

            def anw_an(si, tw=None):
                """AN spikes + bushy currents, cols (b, t) -> Qb columns."""
                t0, W = tw if tw is not None else _win(si)
                e4 = anw_state.pop(si)
                # fp16 hi/lo split of W_bushy: spikes (0/1) and 2^-11 are
                # fp16-exact, so cur_b = Wbh@spk + Wbl_s@(2^-11 spk) matches
                # fp32 up to a 2^-22 residual (verified: 0 spike flips)
                ps_cb = pss.tile([HID, 4 * NGRP * W], dt.float32, tag="win",
                                 bufs=1, name="ps_cb")
                for ch in range(3):
                    an = anp.tile([128, 4 * NGRP * W], dt.float16, tag="an",
                                  name="an")
                    nc.vector.tensor_scalar(an[:, :], e4[:, :],
                                            svt[:, ch:ch + 1], AN_THR,
                                            OP.mult, OP.is_gt)
                    an2 = anp.tile([128, 4 * NGRP * W], dt.float16, tag="an2",
                                   name="an2")
                    nc.vector.tensor_scalar(an2[:, :], an[:, :],
                                            1.0 / 2048.0, None, OP.mult)
                    nc.tensor.matmul(ps_cb[:, :], wbt[2 * ch][:, :], an[:, :],
                                     start=(ch == 0), stop=False)
                    nc.tensor.matmul(ps_cb[:, :], wbt[2 * ch + 1][:, :],
                                     an2[:, :],
                                     start=False, stop=(ch == 2))
                qap = Qb[:, :]
                dest = bass.AP(qap.tensor, qap.offset + t0 + 1,
                               [list(qap.ap[0]), [BST, BLOC], [1, W]])
                nc.scalar.activation(dest, ps_cb[:, :], AF.Copy)

            def tsl(ap2d, tau):
                # strided time-slice: columns b*BST + tau for b in 0..BLOC
                return bass.AP(ap2d.tensor, ap2d.offset + tau,
                               [list(ap2d.ap[0]), [BST, BLOC]])

            def wavefront():
                """Generator: packed LIF wavefront (32 rows), yields per step."""
                m_all, s_all = Mt[:, :], St[:, :]
                m_lo, m_hi = Mt[0:HID, :], Mt[PIC:PTOT, :]
                s_mm = St[0:PIC + HID, :]
                q_all = Qb[:, :]
                qia = {}
                for tau in range(1, NSTEP + 1):
                    nc.vector.scalar_tensor_tensor(
                        tsl(m_lo, tau), tsl(m_lo, tau - 1), BETA,
                        tsl(q_all, tau), OP.mult, OP.add)
                    if tau <= DELTA:
                        nc.vector.tensor_scalar(tsl(m_hi, tau),
                                                tsl(m_hi, tau - 1), BETA,
                                                None, OP.mult)
                    else:
                        qt = qia.pop(tau)
                        nc.vector.scalar_tensor_tensor(
                            tsl(m_hi, tau), tsl(m_hi, tau - 1), BETA,
                            qt[PIC:PTOT, :], OP.mult, OP.add)
                    nc.vector.tensor_scalar(tsl(s_all, tau), tsl(m_all, tau),
                                            THR, None, OP.is_gt)
                    if tau + DELTA <= NSTEP:
                        ps = psn.tile([PTOT, BLOC], dt.float32, tag="snn",
                                      bufs=2, name="ps_snn")
                        nc.tensor.matmul(ps[PIC:PTOT, :], wcat[:, :],
                                         tsl(s_mm, tau),
                                         start=True, stop=True)
                        qt = qip.tile([PTOT, BLOC], dt.float32, tag="qia",
                                      name="qia")
                        qia[tau + DELTA] = qt
                        nc.scalar.activation(qt[PIC:PTOT, :], ps[PIC:PTOT, :],
                                             AF.Copy)
                    nc.vector.tensor_tensor(tsl(m_all, tau), tsl(m_all, tau),
                                            tsl(s_all, tau), OP.subtract)
                    yield

            wf = wavefront()
            emitted = [0]

            def pump(upto):
                upto = min(upto, NSTEP)
                while emitted[0] < upto:
                    next(wf)
                    emitted[0] += 1

            # an_window for strip si-1 is pipelined across the strips of
            # si (se add at g==1, selector matmuls at g==2, AN at g==3) so
            # the PE never waits on the cross-engine envelope chain.
            for si in range(NSTRIP - 1):
                for g in range(NGRP):
                    if si == 0:
                        # narrow pieces: the PE can start after ~0.8us of
                        # DMA instead of 3us, shortening the p-state ramp
                        if g == 0:
                            for p4 in range(4):
                                conv_strip(g, 0, piece=(512 * p4, 512))
                            load_aux_weights()
                        elif g == 1:
                            for p4 in range(4):
                                conv_strip(g, 0, piece=(512 * p4, 512))
                        else:
                            conv_strip(g, 0, piece=(0, 1024))
                            conv_strip(g, 0, piece=(1024, 1024))
                        continue
                    conv_strip(g, si)
                    if si >= 1:
                        if g == 1:
                            anw_se(si - 1)
                        elif g == 2:
                            anw_shf(si - 1)
                        elif g == 3:
                            anw_an(si - 1)
                    # pace wavefront steps unlocked by window si-2 early in
                    # this strip row, window si-1 once anw_an(si-1) ran
                    if g < 4:
                        pump(16 * si - 33 + 4 * (g + 1))
                    else:
                        pump(16 * si - 17 + 4 * (g - 3))
            # strip 7 runs as two piece passes across all groups so the
            # final window's first chunk (t 111..117, needing sample blocks
            # <= 118) overlaps the second piece's conv
            for g in range(NGRP):
                conv_strip(g, 7, piece=(14336, 1024))
                if g == 1:
                    anw_se(6)
                elif g == 2:
                    anw_shf(6)
                elif g == 3:
                    anw_an(6)
                # window 6 currents land at anw_an(6) (g==3); window 5
                # allows tau <= 95 before that
                pump(95 if g < 4 else 95 + 4 * (g - 3))
            anw_se((7, 0), (111, 8))
            anw_shf((7, 0), (111, 8))
            anw_an((7, 0), (111, 8))
            for g in range(NGRP):
                conv_strip(g, 7, piece=(15360, 512))
                pump(min(119, 112 + (g + 1)))
            anw_se((7, 1), (119, 4))
            anw_shf((7, 1), (119, 4))
            anw_an((7, 1), (119, 4))
            for g in range(NGRP):
                conv_strip(g, 7, piece=(15872, 128))
                pump(min(123, 119 + (g + 1)))
            anw_se((7, 2), (123, 1))
            anw_shf((7, 2), (123, 1))
            anw_an((7, 2), (123, 1))
            pump(NSTEP)

            nc.sync.dma_start(out=ospk[:, :], in_=St[PIC + HID:PTOT, :])
            nc.sync.dma_start(out=omem[:, :], in_=Mt[PIC + HID:PTOT, :])

            psn.release()
            pss.release()
            psp.release()
            qip.release()
            e4p.release()
            sep.release()
            anp.release()
            ybp.release()
            hkp.release()

    nc.finalize()
    return nc


def _prep_inputs(audio, gt_kernels, W_bushy, W_ic, W_ac):
    audio = np.ascontiguousarray(audio, dtype=np.float32)
    gt = np.ascontiguousarray(gt_kernels, dtype=np.float32)
    Wb = np.ascontiguousarray(W_bushy, dtype=np.float32)

    gth = gt.astype(np.float16)
    gtl = ((gt - gth.astype(np.float32)) * 2048.0).astype(np.float16)
    lw = np.zeros((6, 128, 128), np.float16)
    for r in range(4):
        sl = slice(r * 32, r * 32 + 32)
        # lhsT[r*32+k, r*32+c] = gt[c, k]; order: {2^11 kh, 2^11 kl, kh}
        # per tap half
        lw[0, sl, sl] = (gth[:, 0:32].astype(np.float32).T * 2048.0
                         ).astype(np.float16)
        lw[1, sl, sl] = gtl[:, 0:32].T
        lw[2, sl, sl] = gth[:, 0:32].T
        lw[3, sl, sl] = (gth[:, 32:64].astype(np.float32).T * 2048.0
                         ).astype(np.float16)
        lw[4, sl, sl] = gtl[:, 32:64].T
        lw[5, sl, sl] = gth[:, 32:64].T

    wb32 = np.zeros((3, 128, HID), np.float32)
    sv = np.zeros((128, 3), np.float32)
    for ch in range(3):
        for u in range(4):
            a = ch * 4 + u
            if a >= ANS:
                continue
            # wb32[ch, u*32+c, h] = W_bushy[h, c*10 + a]
            wb32[ch, u * 32:u * 32 + 32, :] = Wb[:, a::ANS].T
            # AN consumes raw 2^11-scaled two-block sums: fold the /256
            # and the conv 2^-11 into the scale (exact powers of two)
            sv[u * 32:u * 32 + 32, ch] = _SCALES[a] / (256.0 * 2048.0)
    wb = np.zeros((6, 128, HID), np.float16)
    for ch in range(3):
        wbh = wb32[ch].astype(np.float16)
        wb[2 * ch] = wbh
        wb[2 * ch + 1] = ((wb32[ch] - wbh.astype(np.float32)) * 2048.0
                          ).astype(np.float16)
    selr = np.zeros((4, 128, 128), np.float32)
    for r in range(4):
        for u in range(4):
            for c in range(32):
                selr[r, r * 32 + c, u * 32 + c] = 1.0
    # combined IC/AC lhsT: out partitions 0-49 = IC currents from spk_b
    # (spikes at partitions 0-49), 50-59 = AC currents from spk_ic (spikes
    # at partitions 64-113); contraction rows 50-63 are zero
    wca = np.zeros((PIC + HID, HID + OUT), np.float32)
    wca[0:HID, 0:HID] = np.ascontiguousarray(W_ic.T, dtype=np.float32)
    wca[PIC:PIC + HID, HID:HID + OUT] = np.ascontiguousarray(W_ac.T,
                                                             dtype=np.float32)

    apad = np.zeros((B, NPAD), np.float32)
    apad[:, PAD_L:PAD_L + N] = audio
    apadh = apad.astype(np.float16)
    apadl = ((apad - apadh.astype(np.float32)) * 2048.0).astype(np.float16)

    in_maps = []
    for c in range(NCORES):
        rows = slice(c * BLOC, (c + 1) * BLOC)
        in_maps.append({"apadh": apadh[rows], "apadl": apadl[rows],
                        "lw": lw, "wb": wb,
                        "wca": wca, "sv": sv, "selr": selr})
    return in_maps


def kernel(audio, gt_kernels, W_bushy, W_ic, W_ac, _trace=False):
    global _NC_CACHE
    if _NC_CACHE is None:
        _NC_CACHE = _build_nc()
    nc = _NC_CACHE
    in_maps = _prep_inputs(audio, gt_kernels, W_bushy, W_ic, W_ac)
    res = run_bass_kernel_spmd(nc, in_maps, core_ids=list(range(NCORES)),
                               trace=_trace)
    spk = np.empty((B, T, OUT), np.float32)
    mem = np.empty((B, T, OUT), np.float32)
    for c in range(NCORES):
        rs = res.results[c]["ospk"].reshape(OUT, BLOC, BST)
        rm = res.results[c]["omem"].reshape(OUT, BLOC, BST)
        rows = slice(c * BLOC, (c + 1) * BLOC)
        # [o, b, tau] -> [b, t, o] with t = tau - OFF0
        spk[rows] = rs[:, :, OFF0:OFF0 + T].transpose(1, 2, 0)
        mem[rows] = rm[:, :, OFF0:OFF0 + T].transpose(1, 2, 0)
    kernel._last_results = res
    return spk, mem


# revision 34
# speedup vs baseline: 1.0228x; 1.0023x over previous
"""Trainium2 Bass kernel for the gammatone-cochlea + LIF-SNN model.

Pipeline per core (32 of the 256 batch rows, pure data parallel):
  1. Gammatone conv [32ch, 64 taps] via tap-split Hankel matmuls (fp32 PE):
     4 batch rows per 128-partition group, block-diagonal lhsT, two
     accumulating matmuls per 512-sample block (taps 0-31 / 32-63, the
     second reading the same Hankel tile at free offset +32). One combined
     DMA per strip loads the 128-partition Hankel. The strip loop is
     TIME-MAJOR (strip index outer, group inner) so that after strip si
     every batch row's envelope is known for t < 16*si + 15.
  2. ReLU on ScalarE (PSUM -> SBUF copy), then DVE strided block-sums
     (128-sample blocks) into S_all. The /256 framing normalization is
     folded into the AN scales.
  3. Per strip: one strided DVE add forms the two-block sums for the new
     t-window across all 8 groups, selector matmuls replicate channels
     4x across partitions (u,c), and the AN stage (fused tensor_scalar
     mult+is_gt, 3 accumulating matmuls) produces bushy currents which
     land in Q[0:50] at columns b*136 + 1 + t.
  4. SNN: one 32-row packed wavefront. All three LIF layers (bushy 50,
     IC at partitions 64-113, AC at 114-123) update in [124,32] DVE ops:
     scalar_tensor_tensor (beta*mem + cur), is_gt spike, subtract reset.
     IC lags bushy by DELTA steps and AC by 2*DELTA; the per-step PE
     matmul blockdiag(WicT,WacT) @ [spk_b; spk_ic] and its ScalarE
     PSUM->SBUF copy have DELTA steps of slack. Wavefront steps are
     paced through the conv loop, so only ~23 steps remain as tail.
Outputs: spikes/membranes live in partitions 114-123 of the state tiles,
columns b*136 + t + 1 + 2*DELTA; host slices + transposes.
"""
import numpy as np
import concourse.bass as bass
import concourse.bacc as bacc
import concourse.mybir as mybir
import concourse.tile as tile
from concourse.bass_utils import run_bass_kernel_spmd

dt = mybir.dt
AF = mybir.ActivationFunctionType
OP = mybir.AluOpType

NCORES = 8
B, N, C, K = 256, 16000, 32, 64
BLOC = B // NCORES            # 32 batch rows per core
WINDOW, STRIDE, T = 256, 128, 124
ANS, HID, OUT = 10, 50, 10
BETA, THR, AN_THR = 0.95, 1.0, 0.5
PAD_L, PAD_R = 31, 33         # SAME padding for K=64: 31 left, 32 right (+1 slack)
NPAD = PAD_L + N + PAD_R      # 16064
NGRP = BLOC // 4              # 8 groups of 4 rows
NSTRIP = 8
STRIPS = [2048] * 7 + [1664]  # 4-block strips per group

# packed-wavefront SNN layout (partition-sliced ops must start at a
# quadrant boundary, so IC sits at partitions 64-113, AC at 114-123, and
# partitions 50-63 stay zero)
DELTA = 2                     # IC lags bushy by DELTA steps, AC by 2*DELTA
PIC, PTOT = 64, 124           # IC partition base; total SNN partitions
BST = 132                     # per-batch-row column stride (1 + 124 + 2*DELTA <= BST)
FREE = BLOC * BST             # 4352
NSTEP = T + 1 + 2 * DELTA     # wavefront steps tau = 1..NSTEP (130)
OFF0 = 1 + 2 * DELTA          # AC output for t sits at column b*BST + OFF0 + t

# jnp.linspace(0.5, 1.5, 10, dtype=f32), bitexact
_SCALES = np.array([0x3F000000, 0x3F1C71C7, 0x3F38E38E, 0x3F555555, 0x3F71C71D,
                    0x3F871C72, 0x3F955556, 0x3FA38E39, 0x3FB1C71D, 0x3FC00000],
                   dtype=np.uint32).view(np.float32)

_NC_CACHE = None


def _build_nc():
    nc = bacc.Bacc("TRN2", target_bir_lowering=False, debug=False,
                   num_devices=NCORES)

    apadh = nc.dram_tensor("apadh", [BLOC, NPAD], dt.float16,
                           kind="ExternalInput")
    apadl = nc.dram_tensor("apadl", [BLOC, NPAD], dt.float16,
                           kind="ExternalInput")
    lw = nc.dram_tensor("lw", [6, 128, 128], dt.float16, kind="ExternalInput")
    wb = nc.dram_tensor("wb", [6, 128, HID], dt.float16, kind="ExternalInput")
    wca = nc.dram_tensor("wca", [PIC + HID, HID + OUT], dt.float32,
                         kind="ExternalInput")
    sv = nc.dram_tensor("sv", [128, 3], dt.float32, kind="ExternalInput")
    selr = nc.dram_tensor("selr", [4, 128, 128], dt.float32, kind="ExternalInput")
    ospk = nc.dram_tensor("ospk", [OUT, FREE], dt.float32, kind="ExternalOutput")
    omem = nc.dram_tensor("omem", [OUT, FREE], dt.float32, kind="ExternalOutput")

    with tile.TileContext(nc) as tc:
        with tc.tile_pool(name="cpool", bufs=1) as cp:
            # stationary conv operands (fp16): per tap-half {2^11*kh,
            # 2^11*kl, kh}; paired with moving {ah, ah, 2^11*al}
            lwt = [cp.tile([128, 128], dt.float16, name=f"lw{i}")
                   for i in range(6)]
            for i in range(6):
                nc.gpsimd.dma_start(out=lwt[i][:, :], in_=lw[i, :, :])
            svt = cp.tile([128, 3], dt.float32)
            wbt = [cp.tile([128, HID], dt.float16, name=f"wbt{i}") for i in range(6)]
            wcat = cp.tile([PIC + HID, HID + OUT], dt.float32)
            selt = [cp.tile([128, 128], dt.float32, name=f"selt{r}")
                    for r in range(4)]

            def load_aux_weights():
                # deferred until after si=0's Hankel DMAs: none of these are
                # consumed before the si=1 an_window / wavefront stages
                nc.gpsimd.dma_start(out=svt[:, :], in_=sv[:, :])
                for i in range(6):
                    nc.gpsimd.dma_start(out=wbt[i][:, :], in_=wb[i, :, :])
                nc.gpsimd.dma_start(out=wcat[:, :], in_=wca[:, :])
                for r in range(4):
                    nc.gpsimd.dma_start(out=selt[r][:, :], in_=selr[r, :, :])

            S_all = cp.tile([128, NGRP * 126], dt.float32)

            # packed SNN state (32 rows wide)
            Mt = cp.tile([PTOT, FREE], dt.float32, name="Mt")
            St = cp.tile([PTOT, FREE], dt.float32, name="St")
            Qb = cp.tile([HID, FREE], dt.float32, name="Qb")
            nc.gpsimd.memset(Mt[:, :], 0.0)
            nc.gpsimd.memset(St[:, :], 0.0)
            nc.gpsimd.memset(Qb[:, :], 0.0)

            hkp = tc.alloc_tile_pool(name="hkp", bufs=8)
            ybp = tc.alloc_tile_pool(name="ybp", bufs=12)
            anp = tc.alloc_tile_pool(name="anp", bufs=3)
            sep = tc.alloc_tile_pool(name="sep", bufs=3)
            e4p = tc.alloc_tile_pool(name="e4p", bufs=3)
            qip = tc.alloc_tile_pool(name="qip", bufs=DELTA + 3)
            psp = tc.alloc_tile_pool(name="psp", bufs=1, space="PSUM")
            pss = tc.alloc_tile_pool(name="pss", bufs=1, space="PSUM")
            psn = tc.alloc_tile_pool(name="psn", bufs=1, space="PSUM")

            # PE clock warm-up: one throwaway matmul as soon as the first
            # stationary tile lands (~0.4us) starts the HAM p-state ramp,
            # so conv matmuls reach full clock ~2us earlier
            wps = pss.tile([128, 128], dt.float32, tag="win", bufs=1,
                           name="warm")
            nc.tensor.matmul(wps[:, :], lwt[0][:, :], lwt[0][:, :],
                             start=True, stop=True)

            strip_ctr = [0]

            def conv_strip(g, si, piece=None):
                """Conv + framing for rows 4g..4g+4, samples 2048si..+sw."""
                sc = strip_ctr[0]
                strip_ctr[0] += 1
                if piece is None:
                    sw = STRIPS[si]
                    s0 = 2048 * si
                else:
                    s0, sw = piece
                hkh = hkp.tile([128, 2112], dt.float16, tag="hkh", name="hkh")
                hkl = hkp.tile([128, 2112], dt.float16, tag="hkl", name="hkl")
                # Hankel: hk[32*r+k, j] = apad[4g+r, s0 + j + k], one DMA each
                srch = bass.AP(apadh, (4 * g) * NPAD + s0,
                               [[NPAD, 4], [1, 32], [1, sw + 32]])
                nc.sync.dma_start(out=hkh[:, 0:sw + 32], in_=srch)
                srcl = bass.AP(apadl, (4 * g) * NPAD + s0,
                               [[NPAD, 4], [1, 32], [1, sw + 32]])
                nc.sync.dma_start(out=hkl[:, 0:sw + 32], in_=srcl)
                nb4 = (sw + 511) // 512
                accs = []
                # fp16 hi/lo split, 2^11-scaled PSUM: for each tap half,
                # accumulate ah*(2^11 kh) + ah*(2^11 kl) + (2^11 al)*kh;
                # the al*kl term is below fp32 rounding. The 2^-11 is
                # folded into the AN scales (relu commutes with it).
                passes = [(0, hkh, 0), (1, hkh, 0), (2, hkl, 0),
                          (3, hkh, 32), (4, hkh, 32), (5, hkl, 32)]
                for b4 in range(nb4):
                    w = min(512, sw - 512 * b4)
                    acc = psp.tile([128, 512], dt.float32,
                                   tag=f"acc{(4 * sc + b4) % 5}", name="acc")
                    accs.append((acc, w))
                for pi, (li, hkt, off) in enumerate(passes):
                    for b4 in range(nb4):
                        acc, w = accs[b4]
                        nc.tensor.matmul(
                            acc[:, 0:w], lwt[li][:, :],
                            hkt[:, 512 * b4 + off:512 * b4 + off + w],
                            start=(pi == 0), stop=(pi == 5))
                for b4 in range(nb4):
                    acc, w = accs[b4]
                    yb = ybp.tile([128, 512], dt.float32, tag="yb", name="yb")
                    nc.scalar.activation(yb[:, 0:w], acc[:, 0:w], AF.Relu)
                    nblk = w // 128
                    i = s0 // 512 + b4
                    view = bass.AP(yb.tensor, yb.offset,
                                   [list(yb.ap[0]), [128, nblk], [1, 128]])
                    nc.vector.tensor_reduce(
                        S_all[:, g * 126 + 4 * i: g * 126 + 4 * i + nblk],
                        view, axis=mybir.AxisListType.X, op=OP.add)

            def _win(si):
                t0 = max(0, 16 * si - 1)
                t1 = min(T - 1, 16 * si + 14)
                return t0, t1 - t0 + 1

            anw_state = {}

            def anw_se(si, tw=None):
                """Two-block sums for the t-window unlocked by strip si:
                t in [max(0,16si-1), min(123,16si+14)], cols (g, t)."""
                t0, W = tw if tw is not None else _win(si)
                se = sep.tile([128, NGRP * W], dt.float32, tag="senv",
                              name="senv")
                sa = S_all[:, :]
                in0 = bass.AP(sa.tensor, sa.offset + t0,
                              [list(sa.ap[0]), [126, NGRP], [1, W]])
                in1 = bass.AP(sa.tensor, sa.offset + t0 + 1,
                              [list(sa.ap[0]), [126, NGRP], [1, W]])
                nc.vector.tensor_tensor(se[:, :], in0, in1, OP.add)
                anw_state[si] = se

            def anw_shf(si, tw=None):
                """Channel 4x replication via selector matmuls: psum cols
                (r, g, t); Act copy reorders to (g, r, t) = (b, t)."""
                t0, W = tw if tw is not None else _win(si)
                se = anw_state.pop(si)
                shf = pss.tile([128, 4 * NGRP * W], dt.float32, tag="win",
                               bufs=1, name="shf")
                for r in range(4):
                    nc.tensor.matmul(shf[:, r * NGRP * W:(r + 1) * NGRP * W],
                                     selt[r][:, :], se[:, :],
                                     start=True, stop=True)
                e4 = e4p.tile([128, 4 * NGRP * W], dt.float32, tag="e4",
                              name="e4")
                sh = shf[:, :]
                dst = bass.AP(e4.tensor, e4.offset,
                              [list(e4.ap[0]), [W, 4], [4 * W, NGRP], [1, W]])
                nc.scalar.activation(dst, sh, AF.Copy)
                anw_state[si] = e4

            def anw_an(si, tw=None):
                """AN spikes + bushy currents, cols (b, t) -> Qb columns."""
                t0, W = tw if tw is not None else _win(si)
                e4 = anw_state.pop(si)
                # fp16 hi/lo split of W_bushy: spikes (0/1) and 2^-11 are
                # fp16-exact, so cur_b = Wbh@spk + Wbl_s@(2^-11 spk) matches
                # fp32 up to a 2^-22 residual (verified: 0 spike flips)
                ps_cb = pss.tile([HID, 4 * NGRP * W], dt.float32, tag="win",
                                 bufs=1, name="ps_cb")
                for ch in range(3):
                    an = anp.tile([128, 4 * NGRP * W], dt.float16, tag="an",
                                  name="an")
                    nc.vector.tensor_scalar(an[:, :], e4[:, :],
                                            svt[:, ch:ch + 1], AN_THR,
                                            OP.mult, OP.is_gt)
                    an2 = anp.tile([128, 4 * NGRP * W], dt.float16, tag="an2",
                                   name="an2")
                    nc.vector.tensor_scalar(an2[:, :], an[:, :],
                                            1.0 / 2048.0, None, OP.mult)
                    nc.tensor.matmul(ps_cb[:, :], wbt[2 * ch][:, :], an[:, :],
                                     start=(ch == 0), stop=False)
                    nc.tensor.matmul(ps_cb[:, :], wbt[2 * ch + 1][:, :],
                                     an2[:, :],
                                     start=False, stop=(ch == 2))
                qap = Qb[:, :]
                dest = bass.AP(qap.tensor, qap.offset + t0 + 1,
                               [list(qap.ap[0]), [BST, BLOC], [1, W]])
                nc.scalar.activation(dest, ps_cb[:, :], AF.Copy)

            def tsl(ap2d, tau):
                # strided time-slice: columns b*BST + tau for b in 0..BLOC
                return bass.AP(ap2d.tensor, ap2d.offset + tau,
                               [list(ap2d.ap[0]), [BST, BLOC]])

            def wavefront():
                """Generator: packed LIF wavefront (32 rows), yields per step."""
                m_all, s_all = Mt[:, :], St[:, :]
                m_lo, m_hi = Mt[0:HID, :], Mt[PIC:PTOT, :]
                s_mm = St[0:PIC + HID, :]
                q_all = Qb[:, :]
                qia = {}
                for tau in range(1, NSTEP + 1):
                    nc.vector.scalar_tensor_tensor(
                        tsl(m_lo, tau), tsl(m_lo, tau - 1), BETA,
                        tsl(q_all, tau), OP.mult, OP.add)
                    if tau <= DELTA:
                        nc.vector.tensor_scalar(tsl(m_hi, tau),
                                                tsl(m_hi, tau - 1), BETA,
                                                None, OP.mult)
                    else:
                        qt = qia.pop(tau)
                        nc.vector.scalar_tensor_tensor(
                            tsl(m_hi, tau), tsl(m_hi, tau - 1), BETA,
                            qt[PIC:PTOT, :], OP.mult, OP.add)
                    nc.vector.tensor_scalar(tsl(s_all, tau), tsl(m_all, tau),
                                            THR, None, OP.is_gt)
                    if tau + DELTA <= NSTEP:
                        ps = psn.tile([PTOT, BLOC], dt.float32, tag="snn",
                                      bufs=2, name="ps_snn")
                        nc.tensor.matmul(ps[PIC:PTOT, :], wcat[:, :],
                                         tsl(s_mm, tau),
                                         start=True, stop=True)
                        qt = qip.tile([PTOT, BLOC], dt.float32, tag="qia",
                                      name="qia")
                        qia[tau + DELTA] = qt
                        nc.scalar.activation(qt[PIC:PTOT, :], ps[PIC:PTOT, :],
                                             AF.Copy)
                    nc.vector.tensor_tensor(tsl(m_all, tau), tsl(m_all, tau),
                                            tsl(s_all, tau), OP.subtract)
                    yield

            wf = wavefront()
            emitted = [0]

            def pump(upto):
                upto = min(upto, NSTEP)
                while emitted[0] < upto:
                    next(wf)
                    emitted[0] += 1

            # an_window for strip si-1 is pipelined across the strips of
            # si (se add at g==1, selector matmuls at g==2, AN at g==3) so
            # the PE never waits on the cross-engine envelope chain.
            for si in range(NSTRIP - 1):
                for g in range(NGRP):
                    if si == 0:
                        # narrow pieces: the PE can start after ~0.8us of
                        # DMA instead of 3us, shortening the p-state ramp
                        if g == 0:
                            for p4 in range(4):
                                conv_strip(g, 0, piece=(512 * p4, 512))
                            load_aux_weights()
                        elif g == 1:
                            for p4 in range(4):
                                conv_strip(g, 0, piece=(512 * p4, 512))
                        else:
                            conv_strip(g, 0, piece=(0, 1024))
                            conv_strip(g, 0, piece=(1024, 1024))
                        continue
                    conv_strip(g, si)
                    if si >= 1:
                        if g == 1:
                            anw_se(si - 1)
                        elif g == 2:
                            anw_shf(si - 1)
                        elif g == 3:
                            anw_an(si - 1)
                    # pace wavefront steps unlocked by window si-2 early in
                    # this strip row, window si-1 once anw_an(si-1) ran
                    if g < 4:
                        pump(16 * si - 33 + 4 * (g + 1))
                    else:
                        pump(16 * si - 17 + 4 * (g - 3))
            # strip 7 runs as two piece passes across all groups so the
            # final window's first chunk (t 111..117, needing sample blocks
            # <= 118) overlaps the second piece's conv
            for g in range(NGRP):
                conv_strip(g, 7, piece=(14336, 1024))
                if g == 1:
                    anw_se(6)
                elif g == 2:
                    anw_shf(6)
                elif g == 3:
                    anw_an(6)
                # window 6 currents land at anw_an(6) (g==3); window 5
                # allows tau <= 95 before that
                pump(95 if g < 4 else 95 + 4 * (g - 3))
            anw_se((7, 0), (111, 8))
            anw_shf((7, 0), (111, 8))
            anw_an((7, 0), (111, 8))
            for g in range(NGRP):
                conv_strip(g, 7, piece=(15360, 512))
                pump(min(119, 112 + (g + 1)))
            anw_se((7, 1), (119, 4))
            anw_shf((7, 1), (119, 4))
            anw_an((7, 1), (119, 4))
            for g in range(NGRP):
                conv_strip(g, 7, piece=(15872, 128))
                pump(min(123, 119 + (g + 1)))
            anw_se((7, 2), (123, 1))
            anw_shf((7, 2), (123, 1))
            anw_an((7, 2), (123, 1))
            pump(NSTEP)

            nc.sync.dma_start(out=ospk[:, :], in_=St[PIC + HID:PTOT, :])
            nc.sync.dma_start(out=omem[:, :], in_=Mt[PIC + HID:PTOT, :])

            psn.release()
            pss.release()
            psp.release()
            qip.release()
            e4p.release()
            sep.release()
            anp.release()
            ybp.release()
            hkp.release()

    nc.finalize()
    return nc


def _prep_inputs(audio, gt_kernels, W_bushy, W_ic, W_ac):
    audio = np.ascontiguousarray(audio, dtype=np.float32)
    gt = np.ascontiguousarray(gt_kernels, dtype=np.float32)
    Wb = np.ascontiguousarray(W_bushy, dtype=np.float32)

    gth = gt.astype(np.float16)
    gtl = ((gt - gth.astype(np.float32)) * 2048.0).astype(np.float16)
    lw = np.zeros((6, 128, 128), np.float16)
    for r in range(4):
        sl = slice(r * 32, r * 32 + 32)
        # lhsT[r*32+k, r*32+c] = gt[c, k]; order: {2^11 kh, 2^11 kl, kh}
        # per tap half
        lw[0, sl, sl] = (gth[:, 0:32].astype(np.float32).T * 2048.0
                         ).astype(np.float16)
        lw[1, sl, sl] = gtl[:, 0:32].T
        lw[2, sl, sl] = gth[:, 0:32].T
        lw[3, sl, sl] = (gth[:, 32:64].astype(np.float32).T * 2048.0
                         ).astype(np.float16)
        lw[4, sl, sl] = gtl[:, 32:64].T
        lw[5, sl, sl] = gth[:, 32:64].T

    wb32 = np.zeros((3, 128, HID), np.float32)
    sv = np.zeros((128, 3), np.float32)
    for ch in range(3):
        for u in range(4):
            a = ch * 4 + u
            if a >= ANS:
                continue
            # wb32[ch, u*32+c, h] = W_bushy[h, c*10 + a]
            wb32[ch, u * 32:u * 32 + 32, :] = Wb[:, a::ANS].T
            # AN consumes raw 2^11-scaled two-block sums: fold the /256
            # and the conv 2^-11 into the scale (exact powers of two)
            sv[u * 32:u * 32 + 32, ch] = _SCALES[a] / (256.0 * 2048.0)
    wb = np.zeros((6, 128, HID), np.float16)
    for ch in range(3):
        wbh = wb32[ch].astype(np.float16)
        wb[2 * ch] = wbh
        wb[2 * ch + 1] = ((wb32[ch] - wbh.astype(np.float32)) * 2048.0
                          ).astype(np.float16)
    selr = np.zeros((4, 128, 128), np.float32)
    for r in range(4):
        for u in range(4):
            for c in range(32):
                selr[r, r * 32 + c, u * 32 + c] = 1.0
    # combined IC/AC lhsT: out partitions 0-49 = IC currents from spk_b
    # (spikes at partitions 0-49), 50-59 = AC currents from spk_ic (spikes
    # at partitions 64-113); contraction rows 50-63 are zero
    wca = np.zeros((PIC + HID, HID + OUT), np.float32)
    wca[0:HID, 0:HID] = np.ascontiguousarray(W_ic.T, dtype=np.float32)
    wca[PIC:PIC + HID, HID:HID + OUT] = np.ascontiguousarray(W_ac.T,
                                                             dtype=np.float32)

    apad = np.zeros((B, NPAD), np.float32)
    apad[:, PAD_L:PAD_L + N] = audio
    apadh = apad.astype(np.float16)
    apadl = ((apad - apadh.astype(np.float32)) * 2048.0).astype(np.float16)

    in_maps = []
    for c in range(NCORES):
        rows = slice(c * BLOC, (c + 1) * BLOC)
        in_maps.append({"apadh": apadh[rows], "apadl": apadl[rows],
                        "lw": lw, "wb": wb,
                        "wca": wca, "sv": sv, "selr": selr})
    return in_maps


def kernel(audio, gt_kernels, W_bushy, W_ic, W_ac, _trace=False):
    global _NC_CACHE
    if _NC_CACHE is None:
        _NC_CACHE = _build_nc()
    nc = _NC_CACHE
    in_maps = _prep_inputs(audio, gt_kernels, W_bushy, W_ic, W_ac)
    res = run_bass_kernel_spmd(nc, in_maps, core_ids=list(range(NCORES)),
                               trace=_trace)
    spk = np.empty((B, T, OUT), np.float32)
    mem = np.empty((B, T, OUT), np.float32)
    for c in range(NCORES):
        rs = res.results[c]["ospk"].reshape(OUT, BLOC, BST)
        rm = res.results[c]["omem"].reshape(OUT, BLOC, BST)
        rows = slice(c * BLOC, (c + 1) * BLOC)
        # [o, b, tau] -> [b, t, o] with t = tau - OFF0
        spk[rows] = rs[:, :, OFF0:OFF0 + T].transpose(1, 2, 0)
        mem[rows] = rm[:, :, OFF0:OFF0 + T].transpose(1, 2, 0)
    kernel._last_results = res
    return spk, mem
